# revision 1
# baseline (speedup 1.0000x reference)
"""Causal self-attention with anchor-relative rope (ferope), 8-core TRN2 Bass kernel.

Full-scale problem: B=2, T=2048, C=2048, H=16, D=128, M=32.

Sharding (tensor-parallel heads + data-parallel batch):
  - 8 cores = 2 batch groups x 4 cores. Core (b, g) handles batch b, heads 4g..4g+3.
  - qkv projection: each core computes q/k/v only for its heads (w_attn column shard),
    from x[b] transposed (host prep) so the contraction dim c sits on partitions.
  - attention computed with scores transposed: s_T[ki,qi], so both attention
    matmuls contract along partitions with no on-device transposes.
  - y_T head slices ([128, T] per head, c-major) are AllGathered within each
    4-core batch group -> y_all [C, T].
  - output projection is column-sharded: each core computes out[b][:, g*512:(g+1)*512].

All matmuls run as float32r (1 cycle/row at N>=512) except the qkv projection,
whose inputs (xT and w_attn shards) are cast to bf16 on device to fit SBUF.
"""

import math

import numpy as np

import concourse.bass as bass
import concourse.mybir as mybir
import concourse.tile as tile
from concourse import bacc
from concourse.bass_utils import run_bass_kernel_spmd

F32 = mybir.dt.float32
F32R = mybir.dt.float32r
BF16 = mybir.dt.bfloat16

# full-scale dims (hardcoded per harness contract)
B, T, C, H, DH, M = 2, 2048, 2048, 16, 128, 32
N_CORES = 8
GROUPS = 2                     # batch groups
CPG = N_CORES // GROUPS        # cores per group = 4
HPC = H // CPG                 # heads per core = 4
C_LOC = HPC * DH               # 512: per-core head channels
PANEL = 512                    # qi panel width (one psum bank)
KB = 128                       # ki block (partition dim)


def r(ap):
    """View a float32 AP as float32r for full-rate matmul."""
    return ap.bitcast(F32R)


def build_program(T=T, C=C, HPC=HPC, DH=DH, M=M, n_cores=N_CORES, groups=GROUPS):
    """Build the SPMD Bass program (same program on all cores; data differs)."""
    cpg = n_cores // groups
    c_loc = HPC * DH
    n_cb = C // KB            # contraction blocks for qkv/proj
    n_panels = T // PANEL
    n_tb = T // KB
    kb_per_panel = PANEL // KB  # 4
    inv_sqrt_d = 1.0 / math.sqrt(DH)

    nc = bacc.Bacc("TRN2", target_bir_lowering=False, debug=False,
                   num_devices=n_cores)

    xT_d = nc.dram_tensor("xT", [C, T], F32, kind="ExternalInput").ap()
    wqk_d = nc.dram_tensor("wqk", [C, 2 * c_loc], F32, kind="ExternalInput").ap()
    wv_d = nc.dram_tensor("wv", [C, c_loc], F32, kind="ExternalInput").ap()
    wo_d = nc.dram_tensor("wo", [C, c_loc], F32, kind="ExternalInput").ap()
    freqs_d = nc.dram_tensor("freqs", [M], F32, kind="ExternalInput").ap()
    delta_d = nc.dram_tensor("delta", [T], F32, kind="ExternalInput").ap()
    out_d = nc.dram_tensor("out", [T, c_loc], F32, kind="ExternalOutput").ap()

    replica_groups = [list(range(g * cpg, (g + 1) * cpg)) for g in range(groups)]

    with tile.TileContext(nc) as tc:
        with (
            tc.tile_pool(name="dram", bufs=1, space="DRAM") as dram,
            tc.tile_pool(name="const", bufs=1) as const,
            tc.tile_pool(name="qkv", bufs=1) as qkv,
            tc.tile_pool(name="work", bufs=1) as work,
        ):
            y_part = dram.tile([c_loc, T], BF16)
            y_all = dram.tile([cpg * c_loc, T], BF16)

            # ---- constants: trig tables, causal masks, ones ----
            ones128 = const.tile([KB, KB], BF16)
            nc.vector.memset(ones128[:], 1.0)

            sinN = const.tile([2 * M, T], F32)
            cos64 = const.tile([2 * M, T], F32)
            masks = [const.tile([KB, PANEL], BF16, name=f"maskf{p}")
                     for p in range(kb_per_panel)]
            with tc.tile_pool(name="setup", bufs=1) as setup:
                # fr64 = [-freqs; freqs] as per-partition scalars
                fr64 = setup.tile([2 * M, 1], F32)
                nc.sync.dma_start(out=fr64[0:M, :],
                                  in_=freqs_d.rearrange("m -> m ()"))
                nc.sync.dma_start(out=fr64[M:2 * M, :],
                                  in_=freqs_d.rearrange("m -> m ()"))
                nc.vector.tensor_scalar_mul(fr64[0:M, :], fr64[0:M, :], -1.0)

                # delta broadcast across 2M partitions
                delta_row = setup.tile([1, T], F32)
                nc.sync.dma_start(out=delta_row[:],
                                  in_=delta_d.rearrange("t -> () t"))
                delta_rep = setup.tile([2 * M, T], F32)
                nc.gpsimd.partition_broadcast(delta_rep[:], delta_row[:],
                                              channels=2 * M)

                # ang = delta * (+-freqs); sinN = [-sin; sin], cos = [cos; cos]
                ang = setup.tile([2 * M, T], F32)
                nc.vector.tensor_scalar_mul(ang[:], delta_rep[:], fr64[:])
                nc.scalar.activation(sinN[:], ang[:],
                                     mybir.ActivationFunctionType.Sin)
                pi2 = setup.tile([2 * M, 1], F32)
                nc.vector.memset(pi2[:], math.pi / 2)
                nc.scalar.activation(cos64[:], ang[:],
                                     mybir.ActivationFunctionType.Sin,
                                     bias=pi2[:])

                # causal masks for diagonal tiles: mask_p = (qi >= ki + 128*p)
                for p in range(kb_per_panel):
                    mi = setup.tile([KB, PANEL], F32, tag="maski", bufs=2,
                                    name=f"maski{p}")
                    nc.gpsimd.iota(mi[:], pattern=[[1, PANEL]], base=-KB * p,
                                   channel_multiplier=-1,
                                   allow_small_or_imprecise_dtypes=True)
                    nc.vector.tensor_scalar(masks[p][:], mi[:], 0.0, None,
                                            mybir.AluOpType.is_ge)

            # ---- qkv projection for all local heads, single pass over xT ----
            # q/k stored per head as [d, t] bf16; v natural [t, d] bf16.
            q_sb = [qkv.tile([DH, T], BF16, name=f"q{h}") for h in range(HPC)]
            k_sb = [qkv.tile([DH, T], BF16, name=f"k{h}") for h in range(HPC)]
            v_all = qkv.tile([KB, n_tb, c_loc], BF16)
            with tc.tile_pool(name="wload", bufs=1) as wload:
                STAGE_ELEMS = 4 * 512  # f32 staging slot: 8KB/partition

                def load_bf16(dst3, src_t, width, name):
                    """Chunked DRAM->SBUF load of a [C, width] slab (kb-tiled
                    3D view src_t [p, kb, width]) into bf16 tile dst3,
                    chunking along kb so DMA rows stay >=2KB."""
                    kbc = min(max(STAGE_ELEMS // width, 1), n_cb)
                    for ci in range((n_cb + kbc - 1) // kbc):
                        lo = ci * kbc
                        hi = min(lo + kbc, n_cb)
                        st = wload.tile([KB, hi - lo, width], F32,
                                        tag="stage3", bufs=3,
                                        name=f"st_{name}{ci}")
                        nc.sync.dma_start(out=st[:], in_=src_t[:, lo:hi, :])
                        nc.vector.tensor_copy(dst3[:, lo:hi, :], st[:])

                wv_t = wv_d.rearrange("(kb p) c -> p kb c", p=KB)
                wvb3 = wload.tile([KB, n_cb, c_loc], BF16, tag="wvbf")
                load_bf16(wvb3, wv_t, c_loc, "wv")
                wvb = [wvb3[:, i, :] for i in range(n_cb)]
                wqk_t = wqk_d.rearrange("(kb p) c -> p kb c", p=KB)
                wqkb3 = wload.tile([KB, n_cb, 2 * c_loc], BF16, tag="wqkbf")
                load_bf16(wqkb3, wqk_t, 2 * c_loc, "wqk")
                wqkb = [wqkb3[:, i, :] for i in range(n_cb)]
                xT_t = xT_d.rearrange("(kb p) t -> p kb t", p=KB)
                with tc.tile_pool(name="psq", bufs=1, space="PSUM") as psq:
                    for tp in range(n_panels):
                        tps = tp * PANEL
                        xb3 = wload.tile([KB, n_cb, PANEL], BF16, tag="xbf",
                                         bufs=2, name=f"xb{tp}")
                        kbc = min(max(STAGE_ELEMS // PANEL, 1), n_cb)
                        for ci in range((n_cb + kbc - 1) // kbc):
                            lo = ci * kbc
                            hi = min(lo + kbc, n_cb)
                            st = wload.tile([KB, hi - lo, PANEL], F32,
                                            tag="stage3", bufs=3,
                                            name=f"st_x{tp}_{ci}")
                            nc.gpsimd.dma_start(
                                out=st[:],
                                in_=xT_t[:, lo:hi, tps:tps + PANEL])
                            nc.scalar.copy(xb3[:, lo:hi, :], st[:])
                        xbf = [xb3[:, kb, :] for kb in range(n_cb)]
                        # v blocks for the 128-rows inside this panel
                        for tbl in range(kb_per_panel):
                            tb = tp * kb_per_panel + tbl
                            pv = psq.tile([KB, c_loc], F32, tag="v", bufs=3)
                            for kb in range(n_cb):
                                nc.tensor.matmul(
                                    pv[:],
                                    xbf[kb][:, tbl * KB:(tbl + 1) * KB],
                                    wvb[kb],
                                    start=(kb == 0), stop=(kb == n_cb - 1))
                            nc.scalar.copy(v_all[:, tb, :], pv[:])
                        # q/k column blocks: cb<HPC -> q head cb; else k head
                        for cb in range(2 * HPC):
                            pqk = psq.tile([DH, PANEL], F32, tag="qk", bufs=3)
                            for kb in range(n_cb):
                                nc.tensor.matmul(
                                    pqk[:],
                                    wqkb[kb][:, cb * DH:(cb + 1) * DH],
                                    xbf[kb],
                                    start=(kb == 0), stop=(kb == n_cb - 1))
                            dst = q_sb[cb] if cb < HPC else k_sb[cb - HPC]
                            nc.scalar.copy(dst[:, tps:tps + PANEL], pqk[:])

            # ---- rope on rows 0:2M of each q/k head ----
            for u in [t for pair in zip(q_sb, k_sb) for t in pair]:
                sw = work.tile([2 * M, T], BF16, tag="ropesw", bufs=2)
                nc.vector.tensor_copy(sw[0:M, :], u[M:2 * M, :])
                nc.vector.tensor_copy(sw[M:2 * M, :], u[0:M, :])
                nc.vector.tensor_mul(sw[:], sw[:], sinN[:])
                nc.vector.tensor_mul(u[0:2 * M, :], u[0:2 * M, :], cos64[:])
                nc.vector.tensor_add(u[0:2 * M, :], u[0:2 * M, :], sw[:])

            # ---- causal attention per head + per-head AllGather ----
            # y_all rows are head-major: (head, group, d) so each per-head
            # gather writes one contiguous [cpg*DH, T] block
            with tc.tile_pool(name="proj", bufs=1) as proj:
                # prefetch proj weights during attention
                wob = []
                for i in range(n_cb):
                    wo_st = proj.tile([KB, c_loc], F32, tag="wost", bufs=2,
                                      name=f"wost{i}")
                    nc.sync.dma_start(out=wo_st[:],
                                      in_=wo_d[i * KB:(i + 1) * KB, :])
                    wo_sb = proj.tile([KB, c_loc], BF16, tag="wo", bufs=n_cb,
                                      name=f"wo{i}")
                    nc.vector.tensor_copy(wo_sb[:], wo_st[:])
                    wob.append(wo_sb)
                psa_cm = tc.tile_pool(name="psa", bufs=1, space="PSUM")
                psa = psa_cm.__enter__()
                pso_cm = tc.tile_pool(name="pso", bufs=1, space="PSUM")
                pso = pso_cm.__enter__()
                # SBUF f32 accumulators for the output projection
                out_acc = [proj.tile([KB, c_loc], F32, name=f"oacc{i}")
                           for i in range(n_tb)]
                # y_all row (hh, g, p) -> c-block cb = g*HPC + hh
                y_all_tiled = y_all[:].rearrange(
                    "(hh g p) t -> p hh g t", hh=HPC, g=cpg)

                def proj_chunk(hh):
                    """Accumulate head-chunk hh of the output projection;
                    overlaps with later heads' attention + gathers."""
                    for tb in range(n_tb):
                        yt = proj.tile([KB, cpg, KB], BF16, tag="yt", bufs=4)
                        nc.sync.dma_start(
                            out=yt[:],
                            in_=y_all_tiled[:, hh, :, tb * KB:(tb + 1) * KB])
                        po = pso.tile([KB, c_loc], F32, tag="po", bufs=2)
                        for g in range(cpg):
                            nc.tensor.matmul(po[:], yt[:, g, :],
                                             wob[g * HPC + hh][:],
                                             start=(g == 0),
                                             stop=(g == cpg - 1))
                        if hh == 0:
                            nc.vector.tensor_copy(out_acc[tb][:], po[:])
                        else:
                            nc.vector.tensor_add(out_acc[tb][:],
                                                 out_acc[tb][:], po[:])
                        if hh == HPC - 1:
                            nc.sync.dma_start(
                                out=out_d[tb * KB:(tb + 1) * KB, :],
                                in_=out_acc[tb][:])

                for h in range(HPC):
                    qh, kh = q_sb[h], k_sb[h]
                    for J in range(n_panels):
                        nkb = (J + 1) * kb_per_panel
                        py = psa.tile([DH, PANEL], F32, tag="y", bufs=3)
                        pr = psa.tile([KB, PANEL], F32, tag="r", bufs=1)
                        qs = J * PANEL
                        for b in range(nkb):
                            ps = psa.tile([KB, PANEL], F32, tag="s", bufs=2)
                            nc.tensor.matmul(
                                ps[:],
                                kh[:, b * KB:(b + 1) * KB],
                                qh[:, qs:qs + PANEL],
                                start=True, stop=True)
                            et = work.tile([KB, PANEL], BF16, tag="exp",
                                           bufs=4)
                            nc.scalar.activation(
                                et[:], ps[:],
                                mybir.ActivationFunctionType.Exp,
                                scale=inv_sqrt_d)
                            p = b - kb_per_panel * J
                            if p >= 0:
                                nc.vector.tensor_mul(et[:], et[:], masks[p][:])
                            nc.tensor.matmul(
                                py[:],
                                v_all[:, b, h * DH:(h + 1) * DH],
                                et[:],
                                start=(b == 0), stop=(b == nkb - 1))
                            # rowsum, pre-replicated across partitions by
                            # using an all-ones [128,128] stationary operand
                            nc.tensor.matmul(
                                pr[:], ones128[:], et[:],
                                start=(b == 0), stop=(b == nkb - 1))
                        # normalize: y * (1/rowsum)
                        rep = work.tile([KB, PANEL], F32, tag="rep_sb", bufs=2)
                        nc.scalar.copy(rep[:], pr[:])
                        rinv = work.tile([KB, PANEL], F32, tag="rinv", bufs=2)
                        nc.vector.reciprocal_approx_fast(rinv[:], rep[:])
                        ysb = work.tile([DH, PANEL], BF16, tag="ysb", bufs=3)
                        nc.vector.tensor_mul(ysb[:], py[:], rinv[:])
                        nc.sync.dma_start(
                            out=y_part[h * DH:(h + 1) * DH, qs:qs + PANEL],
                            in_=ysb[:])
                    # gather this head's y slice across the batch group
                    nc.gpsimd.collective_compute(
                        "AllGather",
                        mybir.AluOpType.bypass,
                        replica_groups=replica_groups,
                        ins=[y_part[h * DH:(h + 1) * DH, :]],
                        outs=[y_all[h * cpg * DH:(h + 1) * cpg * DH, :]],
                    )
                for h in range(HPC):
                    proj_chunk(h)

                pso_cm.__exit__(None, None, None)
                psa_cm.__exit__(None, None, None)

    nc.compile()
    return nc


def make_in_maps(x, w_attn, w_proj, freqs, delta, n_cores=N_CORES,
                 groups=GROUPS, dh=DH):
    """Host-side sharding: slice/transpose full inputs into per-core maps."""
    x = np.asarray(x, dtype=np.float32)
    w_attn = np.asarray(w_attn, dtype=np.float32)
    w_proj = np.asarray(w_proj, dtype=np.float32)
    freqs = np.asarray(freqs, dtype=np.float32)
    delta = np.asarray(delta, dtype=np.float32)
    b_, t_, c_ = x.shape
    cpg = n_cores // groups
    h_ = w_attn.shape[1] // (3 * dh)
    hpc = h_ // cpg
    c_loc = hpc * dh
    in_maps = []
    for core in range(n_cores):
        g, pos = divmod(core, cpg)
        heads = range(pos * hpc, (pos + 1) * hpc)
        xT = np.ascontiguousarray(x[g].T)
        wqk = np.concatenate(
            [w_attn[:, h * dh:(h + 1) * dh] for h in heads]
            + [w_attn[:, c_ + h * dh:c_ + (h + 1) * dh] for h in heads], axis=1)
        wv = np.concatenate(
            [w_attn[:, 2 * c_ + h * dh:2 * c_ + (h + 1) * dh] for h in heads],
            axis=1)
        wo = np.ascontiguousarray(w_proj[:, pos * c_loc:(pos + 1) * c_loc])
        in_maps.append({
            "xT": xT,
            "wqk": np.ascontiguousarray(wqk),
            "wv": np.ascontiguousarray(wv),
            "wo": wo,
            "freqs": freqs,
            "delta": delta,
        })
    return in_maps


def assemble_output(results, n_cores=N_CORES, groups=GROUPS):
    cpg = n_cores // groups
    outs = []
    for g in range(groups):
        cols = [results[g * cpg + pos]["out"] for pos in range(cpg)]
        outs.append(np.concatenate(cols, axis=1))
    return np.stack(outs, axis=0).astype(np.float32)


_NC_CACHE = {}


def _get_program():
    if "nc" not in _NC_CACHE:
        _NC_CACHE["nc"] = build_program()
    return _NC_CACHE["nc"]


def kernel(x, w_attn, w_proj, freqs, delta):
    nc = _get_program()
    in_maps = make_in_maps(x, w_attn, w_proj, freqs, delta)
    res = run_bass_kernel_spmd(nc, in_maps, list(range(N_CORES)))
    return assemble_output(res.results)



# revision 4
# speedup vs baseline: 1.0566x; 1.0566x over previous
"""Causal self-attention with anchor-relative rope (ferope), 8-core TRN2 Bass kernel.

Full-scale problem: B=2, T=2048, C=2048, H=16, D=128, M=32.

Sharding (tensor-parallel heads + data-parallel batch):
  - 8 cores = 2 batch groups x 4 cores. Core (b, g) handles batch b, heads 4g..4g+3.
  - All matrix inputs are pre-converted to bf16 and pre-permuted on the host so
    each contraction block [128, .] DMAs contiguously into SBUF (no staging).
  - qkv projection: per-core column shard of w_attn; q/k produced in [d, t]
    layout, v in [t, d]; rope applied per 512-panel right after projection.
  - attention with transposed scores s_T[ki, qi]; diagonal blocks narrowed to
    the exact causal triangle; softmax denominator accumulated on the vector
    engine (f32) + one f32r ones-matmul per panel; finalization (normalize +
    AllGather) deferred by one panel to keep the PE from stalling on the DVE.
  - y head/panel blocks AllGathered within each 4-core batch group; output
    projection computed in transposed layout (out^T [c_loc, T]) with PSUM
    accumulation over each head chunk, interleaved between attention heads.
"""

import math

import numpy as np

import concourse.bass as bass
import concourse.mybir as mybir
import concourse.tile as tile
from concourse import bacc
from concourse.bass_utils import run_bass_kernel_spmd

F32 = mybir.dt.float32
F32R = mybir.dt.float32r
BF16 = mybir.dt.bfloat16

# full-scale dims (hardcoded per harness contract)
B, T, C, H, DH, M = 2, 2048, 2048, 16, 128, 32
N_CORES = 8
GROUPS = 2                     # batch groups
CPG = N_CORES // GROUPS        # cores per group = 4
HPC = H // CPG                 # heads per core = 4
C_LOC = HPC * DH               # 512: per-core head channels
PANEL = 512                    # qi panel width (one psum bank)
KB = 128                       # ki block (partition dim)


def r(ap):
    """View a float32 AP as float32r for full-rate matmul."""
    return ap.bitcast(F32R)


def build_program():
    n_cb = C // KB              # 16 contraction blocks for qkv/proj
    n_panels = T // PANEL       # 4
    n_tb = T // KB              # 16
    kb_per_panel = PANEL // KB  # 4
    inv_sqrt_d = 1.0 / math.sqrt(DH)

    nc = bacc.Bacc("TRN2", target_bir_lowering=False, debug=False,
                   num_devices=N_CORES)

    # pre-permuted bf16 inputs: [p, kb, cols] with c = kb*128 + p
    xT_d = nc.dram_tensor("xT", [KB, n_cb, T], BF16, kind="ExternalInput").ap()
    wqk_d = nc.dram_tensor("wqk", [KB, n_cb, 2 * C_LOC], BF16,
                           kind="ExternalInput").ap()
    wv_d = nc.dram_tensor("wv", [KB, n_cb, C_LOC], BF16,
                          kind="ExternalInput").ap()
    wo_d = nc.dram_tensor("wo", [KB, n_cb, C_LOC], BF16,
                          kind="ExternalInput").ap()
    freqs_d = nc.dram_tensor("freqs", [M], F32, kind="ExternalInput").ap()
    delta_d = nc.dram_tensor("delta", [T], F32, kind="ExternalInput").ap()
    # transposed output: host converts back to [T, c_loc]
    outT_d = nc.dram_tensor("outT", [C_LOC, T], F32, kind="ExternalOutput").ap()

    replica_groups = [list(range(g * CPG, (g + 1) * CPG))
                      for g in range(GROUPS)]

    with tile.TileContext(nc) as tc:
        with (
            tc.tile_pool(name="dram", bufs=1, space="DRAM") as dram,
            tc.tile_pool(name="const", bufs=1) as const,
            tc.tile_pool(name="qkv", bufs=1) as qkv,
            tc.tile_pool(name="work", bufs=1) as work,
        ):
            # contiguous [128, 512] blocks per (head, panel) for fine AllGather
            y_part = dram.tile([HPC, n_panels, KB, PANEL], BF16)
            y_all = dram.tile([HPC, n_panels, CPG, KB, PANEL], BF16)

            # ---- constants: trig tables, diag mask, ones ----
            ones128 = const.tile([KB, KB], BF16)
            nc.vector.memset(ones128[:], 1.0)

            sinN = const.tile([2 * M, T], F32)
            cos64 = const.tile([2 * M, T], F32)
            mask128 = const.tile([KB, KB], BF16)
            with tc.tile_pool(name="setup", bufs=1) as setup:
                # fr64 = [-freqs; freqs] as per-partition scalars
                fr64 = setup.tile([2 * M, 1], F32)
                nc.sync.dma_start(out=fr64[0:M, :],
                                  in_=freqs_d.rearrange("m -> m ()"))
                nc.sync.dma_start(out=fr64[M:2 * M, :],
                                  in_=freqs_d.rearrange("m -> m ()"))
                nc.vector.tensor_scalar_mul(fr64[0:M, :], fr64[0:M, :], -1.0)

                # delta broadcast across 2M partitions
                delta_row = setup.tile([1, T], F32)
                nc.sync.dma_start(out=delta_row[:],
                                  in_=delta_d.rearrange("t -> () t"))
                delta_rep = setup.tile([2 * M, T], F32)
                nc.gpsimd.partition_broadcast(delta_rep[:], delta_row[:],
                                              channels=2 * M)

                # ang = delta * (+-freqs); sinN = [-sin; sin], cos = [cos; cos]
                ang = setup.tile([2 * M, T], F32)
                nc.vector.tensor_scalar_mul(ang[:], delta_rep[:], fr64[:])
                nc.scalar.activation(sinN[:], ang[:],
                                     mybir.ActivationFunctionType.Sin)
                pi2 = setup.tile([2 * M, 1], F32)
                nc.vector.memset(pi2[:], math.pi / 2)
                nc.scalar.activation(cos64[:], ang[:],
                                     mybir.ActivationFunctionType.Sin,
                                     bias=pi2[:])

                # diagonal-subblock causal mask: mask[ki, c] = (c >= ki)
                mi = setup.tile([KB, KB], F32)
                nc.gpsimd.iota(mi[:], pattern=[[1, KB]], base=0,
                               channel_multiplier=-1,
                               allow_small_or_imprecise_dtypes=True)
                nc.vector.tensor_scalar(mask128[:], mi[:], 0.0, None,
                                        mybir.AluOpType.is_ge)

            # persistent attention operands
            q_sb = [qkv.tile([DH, T], BF16, name=f"q{h}") for h in range(HPC)]
            k_sb = [qkv.tile([DH, T], BF16, name=f"k{h}") for h in range(HPC)]
            v_all = qkv.tile([KB, n_tb, C_LOC], BF16)

            # ---- qkv projection: direct bf16 loads, per-panel rope ----
            with tc.tile_pool(name="wload", bufs=1) as wload:
                xbf = wload.tile([KB, n_cb, T], BF16)
                wqkb = wload.tile([KB, n_cb, 2 * C_LOC], BF16)
                wvb = wload.tile([KB, n_cb, C_LOC], BF16)
                # interleave loads so early contraction blocks land first;
                # x split per (kb, half) so panel-0 consumers unblock early
                for kb in range(n_cb):
                    nc.sync.dma_start(out=wvb[:, kb, :], in_=wv_d[:, kb, :])
                    nc.sync.dma_start(out=xbf[:, kb, 0:T // 2],
                                      in_=xT_d[:, kb, 0:T // 2])
                    nc.sync.dma_start(out=wqkb[:, kb, :], in_=wqk_d[:, kb, :])
                for kb in range(n_cb):
                    nc.gpsimd.dma_start(out=xbf[:, kb, T // 2:T],
                                        in_=xT_d[:, kb, T // 2:T])

                with tc.tile_pool(name="psq", bufs=1, space="PSUM") as psq:
                    for tp in range(n_panels):
                        tps = tp * PANEL
                        # v blocks for the 128-rows inside this panel
                        for tbl in range(kb_per_panel):
                            tb = tp * kb_per_panel + tbl
                            pv = psq.tile([KB, C_LOC], F32, tag="v", bufs=3)
                            for kb in range(n_cb):
                                nc.tensor.matmul(
                                    pv[:],
                                    xbf[:, kb, tb * KB:(tb + 1) * KB],
                                    wvb[:, kb, :],
                                    start=(kb == 0), stop=(kb == n_cb - 1))
                            nc.scalar.copy(v_all[:, tb, :], pv[:])
                        # q/k column blocks: cb<HPC -> q head cb; else k head
                        for cb in range(2 * HPC):
                            pqk = psq.tile([DH, PANEL], F32, tag="qk", bufs=3)
                            for kb in range(n_cb):
                                nc.tensor.matmul(
                                    pqk[:],
                                    wqkb[:, kb, cb * DH:(cb + 1) * DH],
                                    xbf[:, kb, tps:tps + PANEL],
                                    start=(kb == 0), stop=(kb == n_cb - 1))
                            dst = q_sb[cb] if cb < HPC else k_sb[cb - HPC]
                            nc.scalar.copy(dst[:, tps:tps + PANEL], pqk[:])
                        # rope on rows 0:2M of this panel of each q/k head
                        sl = slice(tps, tps + PANEL)
                        for u in [t for pair in zip(q_sb, k_sb) for t in pair]:
                            sw = work.tile([2 * M, PANEL], BF16, tag="ropesw",
                                           bufs=2)
                            nc.vector.tensor_copy(sw[0:M, :], u[M:2 * M, sl])
                            nc.vector.tensor_copy(sw[M:2 * M, :], u[0:M, sl])
                            nc.vector.tensor_mul(sw[:], sw[:], sinN[:, sl])
                            nc.vector.tensor_mul(u[0:2 * M, sl],
                                                 u[0:2 * M, sl], cos64[:, sl])
                            nc.vector.tensor_add(u[0:2 * M, sl],
                                                 u[0:2 * M, sl], sw[:])

            # ---- attention + interleaved output projection ----
            with tc.tile_pool(name="proj", bufs=1) as proj:
                # prefetch proj weights (overlaps with early attention)
                wo_sb = proj.tile([KB, n_cb, C_LOC], BF16)
                for kb in range(n_cb):
                    nc.sync.dma_start(out=wo_sb[:, kb, :], in_=wo_d[:, kb, :])
                # transposed f32 output accumulators [outc-block][128, T]
                out_acc = [proj.tile([KB, T], F32, name=f"oacc{i}")
                           for i in range(HPC)]

                psa_cm = tc.tile_pool(name="psa", bufs=1, space="PSUM")
                psa = psa_cm.__enter__()

                def attention_panel(h, J):
                    """Emit s/exp/mask/acc/av for panel J of head h; return
                    a finalize closure (rowsum-MM, normalize, DMA, AG)."""
                    qh, kh = q_sb[h], k_sb[h]
                    qs = J * PANEL
                    nkb = (J + 1) * kb_per_panel
                    py = psa.tile([DH, PANEL], F32, tag="y", bufs=2)
                    acc = work.tile([KB, PANEL], F32, tag="acc", bufs=2)
                    for b in range(nkb):
                        p = b - kb_per_panel * J
                        off = max(p, 0) * KB
                        w = PANEL - off
                        ps = psa.tile([KB, PANEL], F32, tag="s", bufs=2)
                        nc.tensor.matmul(
                            ps[:, off:],
                            kh[:, b * KB:(b + 1) * KB],
                            qh[:, qs + off:qs + PANEL],
                            start=True, stop=True)
                        et = work.tile([KB, PANEL], BF16, tag="exp", bufs=4)
                        nc.scalar.activation(
                            et[:, off:], ps[:, off:],
                            mybir.ActivationFunctionType.Exp,
                            scale=inv_sqrt_d)
                        if p >= 0:
                            nc.vector.tensor_mul(et[:, off:off + KB],
                                                 et[:, off:off + KB],
                                                 mask128[:])
                        if b == 0:
                            nc.vector.tensor_copy(acc[:], et[:])
                        else:
                            nc.vector.tensor_add(acc[:, off:], acc[:, off:],
                                                 et[:, off:])
                        nc.tensor.matmul(
                            py[:, off:],
                            v_all[:, b, h * DH:(h + 1) * DH],
                            et[:, off:],
                            start=(b == 0), stop=(b == nkb - 1))

                    def finalize():
                        accb = work.tile([KB, PANEL], BF16, tag="accb", bufs=2)
                        nc.vector.tensor_copy(accb[:], acc[:])
                        pr = psa.tile([KB, PANEL], F32, tag="r", bufs=1)
                        nc.tensor.matmul(pr[:], ones128[:], accb[:],
                                         start=True, stop=True)
                        rinv = work.tile([KB, PANEL], F32, tag="rinv", bufs=2)
                        nc.vector.reciprocal_approx_fast(rinv[:], pr[:])
                        ysb = work.tile([DH, PANEL], BF16, tag="ysb", bufs=3)
                        nc.vector.tensor_mul(ysb[:], py[:], rinv[:])
                        nc.sync.dma_start(out=y_part[h, J], in_=ysb[:])
                        nc.gpsimd.collective_compute(
                            "AllGather",
                            mybir.AluOpType.bypass,
                            replica_groups=replica_groups,
                            ins=[y_part[h, J]],
                            outs=[y_all[h, J]],
                        )
                    return finalize

                def proj_chunk(hh, t_slices):
                    """out^T[:, t] += sum_g wo[(g,hh) block].T @ y[(hh,g), t]
                    for the given t panel indices (AG(hh, J) must be done)."""
                    for J in t_slices:
                        ts_ = J * PANEL
                        ymov = [proj.tile([KB, PANEL], BF16, tag="ymov",
                                          bufs=4, name=f"ym{g}")
                                for g in range(CPG)]
                        for g in range(CPG):
                            nc.sync.dma_start(out=ymov[g][:],
                                              in_=y_all[hh, J, g])
                        for oc in range(HPC):
                            po = psa.tile([KB, PANEL], F32, tag="po", bufs=2)
                            for g in range(CPG):
                                cblk = g * HPC + hh
                                nc.tensor.matmul(
                                    po[:],
                                    wo_sb[:, cblk, oc * KB:(oc + 1) * KB],
                                    ymov[g][:],
                                    start=(g == 0), stop=(g == CPG - 1))
                            if hh == 0:
                                nc.vector.tensor_copy(
                                    out_acc[oc][:, ts_:ts_ + PANEL], po[:])
                            else:
                                nc.vector.tensor_add(
                                    out_acc[oc][:, ts_:ts_ + PANEL],
                                    out_acc[oc][:, ts_:ts_ + PANEL], po[:])
                            if hh == HPC - 1:
                                nc.sync.dma_start(
                                    out=outT_d[oc * KB:(oc + 1) * KB,
                                               ts_:ts_ + PANEL],
                                    in_=out_acc[oc][:, ts_:ts_ + PANEL])

                # software-pipelined: finalize(h, J) emitted after the next
                # panel's matmuls; proj chunk hh emitted mid-head hh+1
                pending = None
                for h in range(HPC):
                    for J in range(n_panels):
                        fin = attention_panel(h, J)
                        if pending is not None:
                            pending()
                        pending = fin
                        if J == 1 and h >= 1:
                            proj_chunk(h - 1, range(n_panels))
                pending()
                # last head: J panels 0..2 gathered during its later panels
                proj_chunk(HPC - 1, range(n_panels))

                psa_cm.__exit__(None, None, None)

    nc.compile()
    return nc


def _perm(a):
    """[C, cols] f32 -> [128, n_cb, cols] bf16 with c = kb*128 + p."""
    import ml_dtypes
    c, cols = a.shape
    return np.ascontiguousarray(
        a.reshape(c // KB, KB, cols).transpose(1, 0, 2)
    ).astype(ml_dtypes.bfloat16)


def make_in_maps(x, w_attn, w_proj, freqs, delta):
    """Host-side sharding: slice/transpose/convert full inputs per core."""
    x = np.asarray(x, dtype=np.float32)
    w_attn = np.asarray(w_attn, dtype=np.float32)
    w_proj = np.asarray(w_proj, dtype=np.float32)
    freqs = np.asarray(freqs, dtype=np.float32)
    delta = np.asarray(delta, dtype=np.float32)
    c_ = x.shape[2]
    in_maps = []
    for core in range(N_CORES):
        g, pos = divmod(core, CPG)
        heads = range(pos * HPC, (pos + 1) * HPC)
        xT = _perm(np.ascontiguousarray(x[g].T))
        wqk = _perm(np.concatenate(
            [w_attn[:, h * DH:(h + 1) * DH] for h in heads]
            + [w_attn[:, c_ + h * DH:c_ + (h + 1) * DH] for h in heads],
            axis=1))
        wv = _perm(np.ascontiguousarray(
            w_attn[:, 2 * c_ + pos * C_LOC:2 * c_ + (pos + 1) * C_LOC]))
        wo = _perm(np.ascontiguousarray(
            w_proj[:, pos * C_LOC:(pos + 1) * C_LOC]))
        in_maps.append({
            "xT": xT, "wqk": wqk, "wv": wv, "wo": wo,
            "freqs": freqs, "delta": delta,
        })
    return in_maps


def assemble_output(results):
    outs = []
    for g in range(GROUPS):
        cols = [results[g * CPG + pos]["outT"].T for pos in range(CPG)]
        outs.append(np.concatenate(cols, axis=1))
    return np.stack(outs, axis=0).astype(np.float32)


_NC_CACHE = {}


def _get_program():
    if "nc" not in _NC_CACHE:
        _NC_CACHE["nc"] = build_program()
    return _NC_CACHE["nc"]


def kernel(x, w_attn, w_proj, freqs, delta):
    nc = _get_program()
    in_maps = make_in_maps(x, w_attn, w_proj, freqs, delta)
    res = run_bass_kernel_spmd(nc, in_maps, list(range(N_CORES)))
    return assemble_output(res.results)


# revision 9
# speedup vs baseline: 1.3099x; 1.2397x over previous
"""Causal self-attention with anchor-relative rope (ferope), 8-core TRN2 Bass kernel.

Full-scale problem: B=2, T=2048, C=2048, H=16, D=128, M=32.

Sharding (tensor-parallel heads + data-parallel batch), collective-free:
  - 8 cores = 2 batch groups x 4 cores. Core (b, g) handles batch b, heads 4g..4g+3.
  - All matrix inputs are pre-converted to bf16 and pre-permuted on the host so
    each contraction block [128, .] DMAs contiguously into SBUF (no staging).
  - qkv projection: per-core column shard of w_attn; q/k produced in [d, t]
    layout, v in [t, d]; rope applied per 512-panel right after projection.
  - attention runs query-panel-outer / head-inner with transposed scores
    s_T[ki, qi]; diagonal blocks narrowed to the exact causal triangle;
    softmax denominator accumulated in bf16 on the vector engine + one
    ones-matmul per panel; finalization deferred one unit to avoid stalls.
  - output projection needs no AllGather: each core computes the full-width
    partial out^T = wo_own^T @ y_own (same flops as a sharded projection,
    contraction over its own 512 channels) and the host sums the 4 partials
    per batch group while unsharding.
"""

import math

import numpy as np

import concourse.bass as bass
import concourse.mybir as mybir
import concourse.tile as tile
from concourse import bacc
from concourse.bass_utils import run_bass_kernel_spmd

F32 = mybir.dt.float32
BF16 = mybir.dt.bfloat16

# full-scale dims (hardcoded per harness contract)
B, T, C, H, DH, M = 2, 2048, 2048, 16, 128, 32
N_CORES = 8
GROUPS = 2                     # batch groups
CPG = N_CORES // GROUPS        # cores per group = 4
HPC = H // CPG                 # heads per core = 4
C_LOC = HPC * DH               # 512: per-core head channels
PANEL = 512                    # qi panel width (one psum bank)
KB = 128                       # ki block (partition dim)


def build_program():
    n_cb = C // KB              # 16 contraction blocks for qkv
    n_oc = C // KB              # 16 output-column blocks for proj
    n_panels = T // PANEL       # 4
    kb_per_panel = PANEL // KB  # 4
    inv_sqrt_d = 1.0 / math.sqrt(DH)

    nc = bacc.Bacc("TRN2", target_bir_lowering=False, debug=False,
                   num_devices=N_CORES)

    # pre-permuted bf16 inputs: [p, kb, cols] with c = kb*128 + p
    xT_d = nc.dram_tensor("xT", [KB, n_cb, T], BF16, kind="ExternalInput").ap()
    wqk_d = nc.dram_tensor("wqk", [KB, n_cb, 2 * C_LOC], BF16,
                           kind="ExternalInput").ap()
    wv_d = nc.dram_tensor("wv", [KB, n_cb, C_LOC], BF16,
                          kind="ExternalInput").ap()
    # proj weight rows for this core's channels: [p, own-cblk, out-cols]
    wo_d = nc.dram_tensor("wo", [KB, HPC, C], BF16, kind="ExternalInput").ap()
    freqs_d = nc.dram_tensor("freqs", [M], F32, kind="ExternalInput").ap()
    delta_d = nc.dram_tensor("delta", [T], F32, kind="ExternalInput").ap()
    # full-width transposed partial projection; host sums over the group
    partT_d = nc.dram_tensor("partT", [C, T], F32, kind="ExternalOutput").ap()

    with tile.TileContext(nc) as tc:
        with (
            tc.tile_pool(name="const", bufs=1) as const,
            tc.tile_pool(name="qkv", bufs=1) as qkv,
            tc.tile_pool(name="work", bufs=1) as work,
        ):
            # ---- constants: trig tables, diag mask, ones ----
            ones128 = const.tile([KB, KB], BF16)
            nc.vector.memset(ones128[:], 1.0)

            sinN = const.tile([2 * M, T], F32)
            cos64 = const.tile([2 * M, T], F32)
            mask128 = const.tile([KB, KB], BF16)
            with tc.tile_pool(name="setup", bufs=1) as setup:
                # fr64 = [-freqs; freqs] as per-partition scalars
                fr64 = setup.tile([2 * M, 1], F32)
                nc.sync.dma_start(out=fr64[0:M, :],
                                  in_=freqs_d.rearrange("m -> m ()"))
                nc.sync.dma_start(out=fr64[M:2 * M, :],
                                  in_=freqs_d.rearrange("m -> m ()"))
                nc.vector.tensor_scalar_mul(fr64[0:M, :], fr64[0:M, :], -1.0)

                # delta broadcast across 2M partitions
                delta_row = setup.tile([1, T], F32)
                nc.sync.dma_start(out=delta_row[:],
                                  in_=delta_d.rearrange("t -> () t"))
                delta_rep = setup.tile([2 * M, T], F32)
                nc.gpsimd.partition_broadcast(delta_rep[:], delta_row[:],
                                              channels=2 * M)

                # ang = delta * (+-freqs); sinN = [-sin; sin], cos = [cos; cos]
                ang = setup.tile([2 * M, T], F32)
                nc.vector.tensor_scalar_mul(ang[:], delta_rep[:], fr64[:])
                nc.scalar.activation(sinN[:], ang[:],
                                     mybir.ActivationFunctionType.Sin)
                pi2 = setup.tile([2 * M, 1], F32)
                nc.vector.memset(pi2[:], math.pi / 2)
                nc.scalar.activation(cos64[:], ang[:],
                                     mybir.ActivationFunctionType.Sin,
                                     bias=pi2[:])

                # diagonal-subblock causal mask: mask[ki, c] = (c >= ki)
                mi = setup.tile([KB, KB], F32)
                nc.gpsimd.iota(mi[:], pattern=[[1, KB]], base=0,
                               channel_multiplier=-1,
                               allow_small_or_imprecise_dtypes=True)
                nc.vector.tensor_scalar(mask128[:], mi[:], 0.0, None,
                                        mybir.AluOpType.is_ge)

            # persistent attention operands
            q_sb = [qkv.tile([DH, T], BF16, name=f"q{h}") for h in range(HPC)]
            k_sb = [qkv.tile([DH, T], BF16, name=f"k{h}") for h in range(HPC)]
            v_all = qkv.tile([KB, T // KB, C_LOC], BF16)

            # ---- qkv projection: direct bf16 loads, per-panel rope ----
            with tc.tile_pool(name="wload", bufs=1) as wload:
                xbf = wload.tile([KB, n_cb, T], BF16)
                wqkb = wload.tile([KB, n_cb, 2 * C_LOC], BF16)
                wvb = wload.tile([KB, n_cb, C_LOC], BF16)
                # priority order: wv + x panels 0/1, then wqk, then the rest
                for kb in range(n_cb):
                    nc.sync.dma_start(out=wvb[:, kb, :], in_=wv_d[:, kb, :])
                    nc.sync.dma_start(out=xbf[:, kb, 0:T // 2],
                                      in_=xT_d[:, kb, 0:T // 2])
                for kb in range(n_cb):
                    nc.sync.dma_start(out=wqkb[:, kb, :], in_=wqk_d[:, kb, :])
                for kb in range(n_cb):
                    nc.gpsimd.dma_start(out=xbf[:, kb, T // 2:T],
                                        in_=xT_d[:, kb, T // 2:T])

                with tc.tile_pool(name="psq", bufs=1, space="PSUM") as psq:
                    for tp in range(n_panels):
                        tps = tp * PANEL
                        # v blocks for the 128-rows inside this panel
                        for tbl in range(kb_per_panel):
                            tb = tp * kb_per_panel + tbl
                            pv = psq.tile([KB, C_LOC], F32, tag="v", bufs=3)
                            for kb in range(n_cb):
                                nc.tensor.matmul(
                                    pv[:],
                                    xbf[:, kb, tb * KB:(tb + 1) * KB],
                                    wvb[:, kb, :],
                                    start=(kb == 0), stop=(kb == n_cb - 1))
                            nc.scalar.copy(v_all[:, tb, :], pv[:])
                        # q/k column blocks: cb<HPC -> q head cb; else k head
                        for cb in range(2 * HPC):
                            pqk = psq.tile([DH, PANEL], F32, tag="qk", bufs=3)
                            for kb in range(n_cb):
                                nc.tensor.matmul(
                                    pqk[:],
                                    wqkb[:, kb, cb * DH:(cb + 1) * DH],
                                    xbf[:, kb, tps:tps + PANEL],
                                    start=(kb == 0), stop=(kb == n_cb - 1))
                            dst = q_sb[cb] if cb < HPC else k_sb[cb - HPC]
                            nc.scalar.copy(dst[:, tps:tps + PANEL], pqk[:])
                        # rope on rows 0:2M of this panel of each q/k head
                        sl = slice(tps, tps + PANEL)
                        for u in [t for pair in zip(q_sb, k_sb) for t in pair]:
                            sw = work.tile([2 * M, PANEL], BF16, tag="ropesw",
                                           bufs=2)
                            nc.vector.tensor_copy(sw[0:M, :], u[M:2 * M, sl])
                            nc.vector.tensor_copy(sw[M:2 * M, :], u[0:M, sl])
                            nc.vector.tensor_mul(sw[:], sw[:], sinN[:, sl])
                            nc.vector.tensor_mul(u[0:2 * M, sl],
                                                 u[0:2 * M, sl], cos64[:, sl])
                            nc.vector.tensor_add(u[0:2 * M, sl],
                                                 u[0:2 * M, sl], sw[:])

            # ---- attention (panel-outer, head-inner) + partial projection ----
            attn_cm = tc.tile_pool(name="attn", bufs=1)
            attn = attn_cm.__enter__()
            ysb_all = attn.tile([DH, HPC, T], BF16)
            wo_sb = attn.tile([KB, HPC, C], BF16)
            for cb in range(HPC):
                nc.gpsimd.dma_start(out=wo_sb[:, cb, :], in_=wo_d[:, cb, :])

            psa_cm = tc.tile_pool(name="psa", bufs=1, space="PSUM")
            psa = psa_cm.__enter__()

            def attention_unit(J, h):
                """Emit s/exp/mask/acc/av for panel J of head h; return the
                deferred finalize closure (rowsum-MM, normalize into ysb)."""
                qh, kh = q_sb[h], k_sb[h]
                qs = J * PANEL
                nkb = (J + 1) * kb_per_panel
                py = psa.tile([DH, PANEL], F32, tag="y", bufs=2)
                acc = work.tile([KB, PANEL], BF16, tag="acc", bufs=2)
                for b in range(nkb):
                    p = b - kb_per_panel * J
                    off = max(p, 0) * KB
                    ps = psa.tile([KB, PANEL], F32, tag="s", bufs=2)
                    nc.tensor.matmul(
                        ps[:, off:],
                        kh[:, b * KB:(b + 1) * KB],
                        qh[:, qs + off:qs + PANEL],
                        start=True, stop=True)
                    et = work.tile([KB, PANEL], BF16, tag="exp", bufs=4)
                    nc.scalar.activation(
                        et[:, off:], ps[:, off:],
                        mybir.ActivationFunctionType.Exp,
                        scale=inv_sqrt_d)
                    if p >= 0:
                        nc.vector.tensor_mul(et[:, off:off + KB],
                                             et[:, off:off + KB], mask128[:])
                    if b == 0:
                        nc.vector.tensor_copy(acc[:], et[:])
                    else:
                        nc.vector.tensor_add(acc[:, off:], acc[:, off:],
                                             et[:, off:])
                    nc.tensor.matmul(
                        py[:, off:],
                        v_all[:, b, h * DH:(h + 1) * DH],
                        et[:, off:],
                        start=(b == 0), stop=(b == nkb - 1))

                def finalize():
                    pr = psa.tile([KB, PANEL], F32, tag="r", bufs=1)
                    nc.tensor.matmul(pr[:], ones128[:], acc[:],
                                     start=True, stop=True)
                    rinv = work.tile([KB, PANEL], F32, tag="rinv", bufs=2)
                    nc.vector.reciprocal_approx_fast(rinv[:], pr[:])
                    nc.vector.tensor_mul(ysb_all[:, h, qs:qs + PANEL],
                                         py[:], rinv[:])
                return finalize

            def proj_panel(J):
                """partT[:, J panel] = sum_h wo[own h].T @ y[h, J panel]."""
                ts_ = J * PANEL
                for oc in range(n_oc):
                    po = psa.tile([KB, PANEL], F32, tag="po", bufs=3)
                    for h in range(HPC):
                        nc.tensor.matmul(
                            po[:],
                            wo_sb[:, h, oc * KB:(oc + 1) * KB],
                            ysb_all[:, h, ts_:ts_ + PANEL],
                            start=(h == 0), stop=(h == HPC - 1))
                    ost = work.tile([KB, PANEL], F32, tag="ost", bufs=4)
                    if oc % 2 == 0:
                        nc.scalar.copy(ost[:], po[:])
                    else:
                        nc.vector.tensor_copy(ost[:], po[:])
                    nc.sync.dma_start(
                        out=partT_d[oc * KB:(oc + 1) * KB, ts_:ts_ + PANEL],
                        in_=ost[:])

            pending = None
            for J in range(n_panels):
                for h in range(HPC):
                    fin = attention_unit(J, h)
                    if pending is not None:
                        pending()
                    pending = fin
                    if h == 1 and J >= 1:
                        proj_panel(J - 1)
            pending()
            proj_panel(n_panels - 1)

            psa_cm.__exit__(None, None, None)
            attn_cm.__exit__(None, None, None)

    nc.compile()
    return nc


def _perm(a):
    """[C, cols] f32 -> [128, n_cb, cols] bf16 with c = kb*128 + p."""
    import ml_dtypes
    c, cols = a.shape
    return np.ascontiguousarray(
        a.reshape(c // KB, KB, cols).transpose(1, 0, 2)
    ).astype(ml_dtypes.bfloat16)


def make_in_maps(x, w_attn, w_proj, freqs, delta):
    """Host-side sharding: slice/transpose/convert full inputs per core."""
    x = np.asarray(x, dtype=np.float32)
    w_attn = np.asarray(w_attn, dtype=np.float32)
    w_proj = np.asarray(w_proj, dtype=np.float32)
    freqs = np.asarray(freqs, dtype=np.float32)
    delta = np.asarray(delta, dtype=np.float32)
    c_ = x.shape[2]
    in_maps = []
    for core in range(N_CORES):
        g, pos = divmod(core, CPG)
        heads = range(pos * HPC, (pos + 1) * HPC)
        xT = _perm(np.ascontiguousarray(x[g].T))
        wqk = _perm(np.concatenate(
            [w_attn[:, h * DH:(h + 1) * DH] for h in heads]
            + [w_attn[:, c_ + h * DH:c_ + (h + 1) * DH] for h in heads],
            axis=1))
        wv = _perm(np.ascontiguousarray(
            w_attn[:, 2 * c_ + pos * C_LOC:2 * c_ + (pos + 1) * C_LOC]))
        wo = _perm(np.ascontiguousarray(
            w_proj[pos * C_LOC:(pos + 1) * C_LOC, :]))
        in_maps.append({
            "xT": xT, "wqk": wqk, "wv": wv, "wo": wo,
            "freqs": freqs, "delta": delta,
        })
    return in_maps


def assemble_output(results):
    outs = []
    for g in range(GROUPS):
        acc = results[g * CPG]["partT"].astype(np.float32)
        for pos in range(1, CPG):
            acc = acc + results[g * CPG + pos]["partT"]
        outs.append(acc.T)
    return np.stack(outs, axis=0).astype(np.float32)


_NC_CACHE = {}


def _get_program():
    if "nc" not in _NC_CACHE:
        _NC_CACHE["nc"] = build_program()
    return _NC_CACHE["nc"]


def kernel(x, w_attn, w_proj, freqs, delta):
    nc = _get_program()
    in_maps = make_in_maps(x, w_attn, w_proj, freqs, delta)
    res = run_bass_kernel_spmd(nc, in_maps, list(range(N_CORES)))
    return assemble_output(res.results)


# revision 15
# speedup vs baseline: 1.5304x; 1.1683x over previous
"""Causal self-attention with anchor-relative rope (ferope), 8-core TRN2 Bass kernel.

Full-scale problem: B=2, T=2048, C=2048, H=16, D=128, M=32.

Sharding (tensor-parallel heads + data-parallel batch), collective-free:
  - 8 cores = 2 batch groups x 4 cores. Core (b, g) handles batch b, heads 4g..4g+3.
  - All matrix inputs are pre-converted to bf16 and pre-permuted on the host so
    each contraction block [128, .] DMAs contiguously into SBUF (no staging).
  - qkv projection: per-core column shard of w_attn; q/k produced in [d, t]
    layout, v in [t, d]; rope applied per 512-panel right after projection.
  - attention runs query-panel-outer / head-inner with transposed scores
    s_T[ki, qi]; diagonal blocks narrowed to the exact causal triangle;
    softmax denominator accumulated in bf16 on the vector engine + one
    ones-matmul per panel; finalization deferred one unit to avoid stalls.
  - output projection needs no AllGather: each core computes the full-width
    partial out^T = wo_own^T @ y_own (same flops as a sharded projection,
    contraction over its own 512 channels) and the host sums the 4 partials
    per batch group while unsharding.
"""

import math

import numpy as np

import concourse.bass as bass
import concourse.mybir as mybir
import concourse.tile as tile
from concourse import bacc
from concourse.bass_utils import run_bass_kernel_spmd

F32 = mybir.dt.float32
BF16 = mybir.dt.bfloat16

# full-scale dims (hardcoded per harness contract)
B, T, C, H, DH, M = 2, 2048, 2048, 16, 128, 32
N_CORES = 8
GROUPS = 2                     # batch groups
CPG = N_CORES // GROUPS        # cores per group = 4
HPC = H // CPG                 # heads per core = 4
C_LOC = HPC * DH               # 512: per-core head channels
PANEL = 512                    # qi panel width (one psum bank)
KB = 128                       # ki block (partition dim)


def build_program():
    n_cb = C // KB              # 16 contraction blocks for qkv
    n_oc = C // KB              # 16 output-column blocks for proj
    n_panels = T // PANEL       # 4
    kb_per_panel = PANEL // KB  # 4
    inv_sqrt_d = 1.0 / math.sqrt(DH)

    nc = bacc.Bacc("TRN2", target_bir_lowering=False, debug=False,
                   num_devices=N_CORES)

    # pre-permuted bf16 inputs, contiguous per partition row so each tensor
    # loads with one big-descriptor DMA. x is quarter-major: [qt, p, kb, 512]
    xT_d = nc.dram_tensor("xT", [n_panels, KB, n_cb, PANEL], BF16,
                          kind="ExternalInput").ap()
    wqk_d = nc.dram_tensor("wqk", [KB, n_cb, 2 * C_LOC], BF16,
                           kind="ExternalInput").ap()
    wv_d = nc.dram_tensor("wv", [KB, n_cb, C_LOC], BF16,
                          kind="ExternalInput").ap()
    # proj weight rows for this core's channels: [p, own-cblk, out-cols]
    wo_d = nc.dram_tensor("wo", [KB, HPC, C], BF16, kind="ExternalInput").ap()
    freqs_d = nc.dram_tensor("freqs", [M], F32, kind="ExternalInput").ap()
    delta_d = nc.dram_tensor("delta", [T], F32, kind="ExternalInput").ap()
    # full-width transposed partial projection; host sums over the group
    partT_d = nc.dram_tensor("partT", [C, T], F32, kind="ExternalOutput").ap()

    with tile.TileContext(nc) as tc:
        with (
            tc.tile_pool(name="const", bufs=1) as const,
            tc.tile_pool(name="qkv", bufs=1) as qkv,
            tc.tile_pool(name="work", bufs=1) as work,
        ):
            # persistent attention operands
            q_sb = [qkv.tile([DH, T], BF16, name=f"q{h}") for h in range(HPC)]
            k_sb = [qkv.tile([DH, T], BF16, name=f"k{h}") for h in range(HPC)]
            v_all = qkv.tile([KB, T // KB, C_LOC], BF16)

            ones128 = const.tile([KB, KB], BF16)
            sinN = const.tile([2 * M, T], F32)
            cos64 = const.tile([2 * M, T], F32)
            mask128 = const.tile([KB, KB], BF16)

            # ---- qkv projection: direct bf16 loads, per-panel rope ----
            with tc.tile_pool(name="wload", bufs=1) as wload:
                # issue the big loads FIRST (one contiguous DMA per tensor,
                # x per quarter) so nothing gates the first matmul chain
                xbf = wload.tile([KB, n_panels, n_cb, PANEL], BF16)
                wqkb = wload.tile([KB, n_cb, 2 * C_LOC], BF16)
                wvb = wload.tile([KB, n_cb, C_LOC], BF16)
                nc.sync.dma_start(out=wvb[:], in_=wv_d)
                nc.sync.dma_start(
                    out=xbf[:, 0], in_=xT_d[0])
                nc.scalar.dma_start(out=wqkb[:], in_=wqk_d)
                nc.scalar.dma_start(
                    out=xbf[:, 1], in_=xT_d[1])
                for qt in (2, 3):
                    nc.gpsimd.dma_start(
                        out=xbf[:, qt],
                        in_=xT_d[qt])

                # ---- constants: trig tables, diag mask, ones ----
                nc.vector.memset(ones128[:], 1.0)
                with tc.tile_pool(name="setup", bufs=1) as setup:
                    # fr64 = [-freqs; freqs] as per-partition scalars
                    fr64 = setup.tile([2 * M, 1], F32)
                    nc.sync.dma_start(out=fr64[0:M, :],
                                      in_=freqs_d.rearrange("m -> m ()"))
                    nc.sync.dma_start(out=fr64[M:2 * M, :],
                                      in_=freqs_d.rearrange("m -> m ()"))
                    nc.vector.tensor_scalar_mul(fr64[0:M, :], fr64[0:M, :],
                                                -1.0)

                    # delta replicated across 2M partitions via 0-stride DMA
                    delta_rep = setup.tile([2 * M, T], F32)
                    nc.sync.dma_start(
                        out=delta_rep[:],
                        in_=delta_d.rearrange("t -> () t")
                        .partition_broadcast(2 * M))

                    # ang = delta * (+-freqs) in place; sinN/cos via Sin
                    nc.vector.tensor_scalar_mul(delta_rep[:], delta_rep[:],
                                                fr64[:])
                    nc.scalar.activation(sinN[:], delta_rep[:],
                                         mybir.ActivationFunctionType.Sin)
                    pi2 = setup.tile([2 * M, 1], F32)
                    nc.vector.memset(pi2[:], math.pi / 2)
                    nc.scalar.activation(cos64[:], delta_rep[:],
                                         mybir.ActivationFunctionType.Sin,
                                         bias=pi2[:])

                    # diagonal-subblock causal mask: mask[ki, c] = (c >= ki)
                    mi = setup.tile([KB, KB], F32)
                    nc.gpsimd.iota(mi[:], pattern=[[1, KB]], base=0,
                                   channel_multiplier=-1,
                                   allow_small_or_imprecise_dtypes=True)
                    nc.vector.tensor_scalar(mask128[:], mi[:], 0.0, None,
                                            mybir.AluOpType.is_ge)

                with tc.tile_pool(name="psq", bufs=1, space="PSUM") as psq:
                    for tp in range(n_panels):
                        tps = tp * PANEL
                        # v blocks for the 128-rows inside this panel
                        for tbl in range(kb_per_panel):
                            tb = tp * kb_per_panel + tbl
                            pv = psq.tile([KB, C_LOC], F32, tag="v", bufs=3)
                            for kb in range(n_cb):
                                nc.tensor.matmul(
                                    pv[:],
                                    xbf[:, tp, kb, tbl * KB:(tbl + 1) * KB],
                                    wvb[:, kb, :],
                                    start=(kb == 0), stop=(kb == n_cb - 1))
                            nc.scalar.copy(v_all[:, tb, :], pv[:])
                        # q/k column blocks: cb<HPC -> q head cb; else k head
                        for cb in range(2 * HPC):
                            pqk = psq.tile([DH, PANEL], F32, tag="qk", bufs=3)
                            for kb in range(n_cb):
                                nc.tensor.matmul(
                                    pqk[:],
                                    wqkb[:, kb, cb * DH:(cb + 1) * DH],
                                    xbf[:, tp, kb, :],
                                    start=(kb == 0), stop=(kb == n_cb - 1))
                            dst = q_sb[cb] if cb < HPC else k_sb[cb - HPC]
                            nc.scalar.copy(dst[:, tps:tps + PANEL], pqk[:])
                        # rope on rows 0:2M of this panel of each q/k head
                        sl = slice(tps, tps + PANEL)
                        for u in [t for pair in zip(q_sb, k_sb) for t in pair]:
                            sw = work.tile([2 * M, PANEL], BF16, tag="ropesw",
                                           bufs=2)
                            nc.vector.tensor_copy(sw[0:M, :], u[M:2 * M, sl])
                            nc.vector.tensor_copy(sw[M:2 * M, :], u[0:M, sl])
                            nc.vector.tensor_mul(sw[:], sw[:], sinN[:, sl])
                            nc.vector.tensor_mul(u[0:2 * M, sl],
                                                 u[0:2 * M, sl], cos64[:, sl])
                            nc.vector.tensor_add(u[0:2 * M, sl],
                                                 u[0:2 * M, sl], sw[:])

            # ---- attention (panel-outer, head-inner) + partial projection ----
            attn_cm = tc.tile_pool(name="attn", bufs=1)
            attn = attn_cm.__enter__()
            ysb_all = attn.tile([DH, HPC, T], BF16)
            wo_sb = attn.tile([KB, HPC, C], BF16)
            for cb in range(HPC):
                nc.gpsimd.dma_start(out=wo_sb[:, cb, :], in_=wo_d[:, cb, :])

            psa_cm = tc.tile_pool(name="psa", bufs=1, space="PSUM")
            psa = psa_cm.__enter__()

            def attention_unit(J, h):
                """Emit s/exp/mask/acc/av for panel J of head h; return the
                deferred finalize closure (rowsum-MM, normalize into ysb)."""
                qh, kh = q_sb[h], k_sb[h]
                qs = J * PANEL
                nkb = (J + 1) * kb_per_panel
                py = psa.tile([DH, PANEL], F32, tag="y", bufs=2)
                acc = work.tile([KB, PANEL], BF16, tag="acc", bufs=2)
                for b in range(nkb):
                    p = b - kb_per_panel * J
                    off = max(p, 0) * KB
                    ps = psa.tile([KB, PANEL], F32, tag="s", bufs=2)
                    nc.tensor.matmul(
                        ps[:, off:],
                        kh[:, b * KB:(b + 1) * KB],
                        qh[:, qs + off:qs + PANEL],
                        start=True, stop=True)
                    et = work.tile([KB, PANEL], BF16, tag="exp", bufs=4)
                    nc.scalar.activation(
                        et[:, off:], ps[:, off:],
                        mybir.ActivationFunctionType.Exp,
                        scale=inv_sqrt_d)
                    if p >= 0:
                        nc.vector.tensor_mul(et[:, off:off + KB],
                                             et[:, off:off + KB], mask128[:])
                    if b == 0:
                        nc.vector.tensor_copy(acc[:], et[:])
                    else:
                        nc.vector.tensor_add(acc[:, off:], acc[:, off:],
                                             et[:, off:])
                    nc.tensor.matmul(
                        py[:, off:],
                        v_all[:, b, h * DH:(h + 1) * DH],
                        et[:, off:],
                        start=(b == 0), stop=(b == nkb - 1))

                def finalize():
                    pr = psa.tile([KB, PANEL], F32, tag="r", bufs=1)
                    nc.tensor.matmul(pr[:], ones128[:], acc[:],
                                     start=True, stop=True)
                    rinv = work.tile([KB, PANEL], F32, tag="rinv", bufs=2)
                    nc.vector.reciprocal_approx_fast(rinv[:], pr[:])
                    nc.vector.tensor_mul(ysb_all[:, h, qs:qs + PANEL],
                                         py[:], rinv[:])
                return finalize

            def proj_panel(J):
                """partT[:, J panel] = sum_h wo[own h].T @ y[h, J panel]."""
                ts_ = J * PANEL
                for oc in range(n_oc):
                    po = psa.tile([KB, PANEL], F32, tag="po", bufs=3)
                    for h in range(HPC):
                        nc.tensor.matmul(
                            po[:],
                            wo_sb[:, h, oc * KB:(oc + 1) * KB],
                            ysb_all[:, h, ts_:ts_ + PANEL],
                            start=(h == 0), stop=(h == HPC - 1))
                    ost = work.tile([KB, PANEL], F32, tag="ost", bufs=4)
                    if oc % 2 == 0:
                        nc.scalar.copy(ost[:], po[:])
                    else:
                        nc.vector.tensor_copy(ost[:], po[:])
                    nc.sync.dma_start(
                        out=partT_d[oc * KB:(oc + 1) * KB, ts_:ts_ + PANEL],
                        in_=ost[:])

            pending = None
            for J in range(n_panels):
                for h in range(HPC):
                    fin = attention_unit(J, h)
                    if pending is not None:
                        pending()
                    pending = fin
                    if h == 1 and J >= 1:
                        proj_panel(J - 1)
            pending()
            proj_panel(n_panels - 1)

            psa_cm.__exit__(None, None, None)
            attn_cm.__exit__(None, None, None)

    nc.compile()
    return nc


def _perm(a):
    """[C, cols] f32 -> [128, n_cb, cols] bf16 with c = kb*128 + p."""
    import ml_dtypes
    c, cols = a.shape
    return np.ascontiguousarray(
        a.reshape(c // KB, KB, cols).transpose(1, 0, 2)
    ).astype(ml_dtypes.bfloat16)


def make_in_maps(x, w_attn, w_proj, freqs, delta):
    """Host-side sharding: slice/transpose/convert full inputs per core."""
    x = np.asarray(x, dtype=np.float32)
    w_attn = np.asarray(w_attn, dtype=np.float32)
    w_proj = np.asarray(w_proj, dtype=np.float32)
    freqs = np.asarray(freqs, dtype=np.float32)
    delta = np.asarray(delta, dtype=np.float32)
    c_ = x.shape[2]
    in_maps = []
    for core in range(N_CORES):
        g, pos = divmod(core, CPG)
        heads = range(pos * HPC, (pos + 1) * HPC)
        # [qt, p, kb, 512] with c = kb*128 + p, t = qt*512 + tq
        xT = np.ascontiguousarray(
            _perm(np.ascontiguousarray(x[g].T))
            .reshape(KB, C // KB, T // PANEL, PANEL).transpose(2, 0, 1, 3))
        wqk = _perm(np.concatenate(
            [w_attn[:, h * DH:(h + 1) * DH] for h in heads]
            + [w_attn[:, c_ + h * DH:c_ + (h + 1) * DH] for h in heads],
            axis=1))
        wv = _perm(np.ascontiguousarray(
            w_attn[:, 2 * c_ + pos * C_LOC:2 * c_ + (pos + 1) * C_LOC]))
        wo = _perm(np.ascontiguousarray(
            w_proj[pos * C_LOC:(pos + 1) * C_LOC, :]))
        in_maps.append({
            "xT": xT, "wqk": wqk, "wv": wv, "wo": wo,
            "freqs": freqs, "delta": delta,
        })
    return in_maps


def assemble_output(results):
    outs = []
    for g in range(GROUPS):
        acc = results[g * CPG]["partT"].astype(np.float32)
        for pos in range(1, CPG):
            acc = acc + results[g * CPG + pos]["partT"]
        outs.append(acc.T)
    return np.stack(outs, axis=0).astype(np.float32)


_NC_CACHE = {}


def _get_program():
    if "nc" not in _NC_CACHE:
        _NC_CACHE["nc"] = build_program()
    return _NC_CACHE["nc"]


def kernel(x, w_attn, w_proj, freqs, delta):
    nc = _get_program()
    in_maps = make_in_maps(x, w_attn, w_proj, freqs, delta)
    res = run_bass_kernel_spmd(nc, in_maps, list(range(N_CORES)))
    return assemble_output(res.results)


# revision 17
# speedup vs baseline: 1.5636x; 1.0217x over previous
"""Causal self-attention with anchor-relative rope (ferope), 8-core TRN2 Bass kernel.

Full-scale problem: B=2, T=2048, C=2048, H=16, D=128, M=32.

Sharding (tensor-parallel heads + data-parallel batch), collective-free:
  - 8 cores = 2 batch groups x 4 cores. Core (b, g) handles batch b, heads 4g..4g+3.
  - All matrix inputs are pre-converted to bf16 and pre-permuted on the host so
    each contraction block [128, .] DMAs contiguously into SBUF (no staging).
  - qkv projection: per-core column shard of w_attn; q/k produced in [d, t]
    layout, v in [t, d]; rope applied per 512-panel right after projection.
  - attention runs query-panel-outer / head-inner with transposed scores
    s_T[ki, qi]; diagonal blocks narrowed to the exact causal triangle;
    softmax denominator accumulated in bf16 on the vector engine + one
    ones-matmul per panel; finalization deferred one unit to avoid stalls.
  - output projection needs no AllGather: each core computes the full-width
    partial out^T = wo_own^T @ y_own (same flops as a sharded projection,
    contraction over its own 512 channels) and the host sums the 4 partials
    per batch group while unsharding.
"""

import math

import numpy as np

import concourse.bass as bass
import concourse.mybir as mybir
import concourse.tile as tile
from concourse import bacc
from concourse.bass_utils import run_bass_kernel_spmd

F32 = mybir.dt.float32
BF16 = mybir.dt.bfloat16

# full-scale dims (hardcoded per harness contract)
B, T, C, H, DH, M = 2, 2048, 2048, 16, 128, 32
N_CORES = 8
GROUPS = 2                     # batch groups
CPG = N_CORES // GROUPS        # cores per group = 4
HPC = H // CPG                 # heads per core = 4
C_LOC = HPC * DH               # 512: per-core head channels
PANEL = 512                    # qi panel width (one psum bank)
KB = 128                       # ki block (partition dim)


def build_program():
    n_cb = C // KB              # 16 contraction blocks for qkv
    n_oc = C // KB              # 16 output-column blocks for proj
    n_panels = T // PANEL       # 4
    kb_per_panel = PANEL // KB  # 4
    inv_sqrt_d = 1.0 / math.sqrt(DH)

    nc = bacc.Bacc("TRN2", target_bir_lowering=False, debug=False,
                   num_devices=N_CORES)

    # pre-permuted bf16 inputs, contiguous per partition row so each tensor
    # loads with one big-descriptor DMA. x is quarter-major: [qt, p, kb, 512]
    xT_d = nc.dram_tensor("xT", [n_panels, KB, n_cb, PANEL], BF16,
                          kind="ExternalInput").ap()
    wqk_d = nc.dram_tensor("wqk", [KB, n_cb, 2 * C_LOC], BF16,
                           kind="ExternalInput").ap()
    wv_d = nc.dram_tensor("wv", [KB, n_cb, C_LOC], BF16,
                          kind="ExternalInput").ap()
    # proj weight rows for this core's channels: [p, own-cblk, out-cols]
    wo_d = nc.dram_tensor("wo", [KB, HPC, C], BF16, kind="ExternalInput").ap()
    freqs_d = nc.dram_tensor("freqs", [M], F32, kind="ExternalInput").ap()
    delta_d = nc.dram_tensor("delta", [T], F32, kind="ExternalInput").ap()
    # full-width transposed partial projection; host sums over the group
    partT_d = nc.dram_tensor("partT", [C, T], F32, kind="ExternalOutput").ap()

    with tile.TileContext(nc) as tc:
        with (
            tc.tile_pool(name="const", bufs=1) as const,
            tc.tile_pool(name="qkv", bufs=1) as qkv,
            tc.tile_pool(name="work", bufs=1) as work,
        ):
            # persistent attention operands
            q_sb = [qkv.tile([DH, T], BF16, name=f"q{h}") for h in range(HPC)]
            k_sb = [qkv.tile([DH, T], BF16, name=f"k{h}") for h in range(HPC)]
            v_all = qkv.tile([KB, T // KB, C_LOC], BF16)

            ones128 = const.tile([KB, KB], BF16)
            sinN = const.tile([2 * M, T], F32)
            cos64 = const.tile([2 * M, T], F32)
            mask128 = const.tile([KB, KB], BF16)

            # ---- qkv projection: direct bf16 loads, per-panel rope ----
            with tc.tile_pool(name="wload", bufs=1) as wload:
                # issue the big loads FIRST (one contiguous DMA per tensor,
                # x per quarter) so nothing gates the first matmul chain
                xbf = wload.tile([KB, n_panels, n_cb, PANEL], BF16)
                wqkb = wload.tile([KB, n_cb, 2 * C_LOC], BF16)
                wvb = wload.tile([KB, n_cb, C_LOC], BF16)
                # chunk along kb (full 128 partitions per chunk) issued in
                # priority order so the first matmul chains' inputs land
                # early instead of together with everything else
                for k4 in range(0, n_cb, 4):
                    ks = slice(k4, k4 + 4)
                    nc.sync.dma_start(out=wvb[:, ks, :], in_=wv_d[:, ks, :])
                    nc.sync.dma_start(out=xbf[:, 0, ks, :],
                                      in_=xT_d[0][:, ks, :])
                for k4 in range(0, n_cb, 4):
                    ks = slice(k4, k4 + 4)
                    nc.scalar.dma_start(out=wqkb[:, ks, :],
                                        in_=wqk_d[:, ks, :])
                    nc.scalar.dma_start(out=xbf[:, 1, ks, :],
                                        in_=xT_d[1][:, ks, :])
                for qt in (2, 3):
                    for k4 in range(0, n_cb, 4):
                        ks = slice(k4, k4 + 4)
                        nc.gpsimd.dma_start(out=xbf[:, qt, ks, :],
                                            in_=xT_d[qt][:, ks, :])

                # ---- constants: trig tables, diag mask, ones ----
                nc.vector.memset(ones128[:], 1.0)
                with tc.tile_pool(name="setup", bufs=1) as setup:
                    # fr64 = [-freqs; freqs] as per-partition scalars
                    fr64 = setup.tile([2 * M, 1], F32)
                    nc.sync.dma_start(out=fr64[0:M, :],
                                      in_=freqs_d.rearrange("m -> m ()"))
                    nc.sync.dma_start(out=fr64[M:2 * M, :],
                                      in_=freqs_d.rearrange("m -> m ()"))
                    nc.vector.tensor_scalar_mul(fr64[0:M, :], fr64[0:M, :],
                                                -1.0)

                    # delta replicated across 2M partitions via 0-stride DMA
                    delta_rep = setup.tile([2 * M, T], F32)
                    nc.sync.dma_start(
                        out=delta_rep[:],
                        in_=delta_d.rearrange("t -> () t")
                        .partition_broadcast(2 * M))

                    # ang = delta * (+-freqs) in place; sinN/cos via Sin
                    nc.vector.tensor_scalar_mul(delta_rep[:], delta_rep[:],
                                                fr64[:])
                    nc.scalar.activation(sinN[:], delta_rep[:],
                                         mybir.ActivationFunctionType.Sin)
                    pi2 = setup.tile([2 * M, 1], F32)
                    nc.vector.memset(pi2[:], math.pi / 2)
                    nc.scalar.activation(cos64[:], delta_rep[:],
                                         mybir.ActivationFunctionType.Sin,
                                         bias=pi2[:])

                    # diagonal-subblock causal mask: mask[ki, c] = (c >= ki)
                    mi = setup.tile([KB, KB], F32)
                    nc.gpsimd.iota(mi[:], pattern=[[1, KB]], base=0,
                                   channel_multiplier=-1,
                                   allow_small_or_imprecise_dtypes=True)
                    nc.vector.tensor_scalar(mask128[:], mi[:], 0.0, None,
                                            mybir.AluOpType.is_ge)

                with tc.tile_pool(name="psq", bufs=1, space="PSUM") as psq:
                    for tp in range(n_panels):
                        tps = tp * PANEL
                        # v blocks for the 128-rows inside this panel
                        for tbl in range(kb_per_panel):
                            tb = tp * kb_per_panel + tbl
                            pv = psq.tile([KB, C_LOC], F32, tag="v", bufs=3)
                            for kb in range(n_cb):
                                nc.tensor.matmul(
                                    pv[:],
                                    xbf[:, tp, kb, tbl * KB:(tbl + 1) * KB],
                                    wvb[:, kb, :],
                                    start=(kb == 0), stop=(kb == n_cb - 1))
                            nc.scalar.copy(v_all[:, tb, :], pv[:])
                        # q/k column blocks: cb<HPC -> q head cb; else k head
                        for cb in range(2 * HPC):
                            pqk = psq.tile([DH, PANEL], F32, tag="qk", bufs=3)
                            for kb in range(n_cb):
                                nc.tensor.matmul(
                                    pqk[:],
                                    wqkb[:, kb, cb * DH:(cb + 1) * DH],
                                    xbf[:, tp, kb, :],
                                    start=(kb == 0), stop=(kb == n_cb - 1))
                            dst = q_sb[cb] if cb < HPC else k_sb[cb - HPC]
                            nc.scalar.copy(dst[:, tps:tps + PANEL], pqk[:])
                        # rope on rows 0:2M of this panel of each q/k head
                        sl = slice(tps, tps + PANEL)
                        for u in [t for pair in zip(q_sb, k_sb) for t in pair]:
                            sw = work.tile([2 * M, PANEL], BF16, tag="ropesw",
                                           bufs=2)
                            nc.vector.tensor_copy(sw[0:M, :], u[M:2 * M, sl])
                            nc.vector.tensor_copy(sw[M:2 * M, :], u[0:M, sl])
                            nc.vector.tensor_mul(sw[:], sw[:], sinN[:, sl])
                            nc.vector.tensor_mul(u[0:2 * M, sl],
                                                 u[0:2 * M, sl], cos64[:, sl])
                            nc.vector.tensor_add(u[0:2 * M, sl],
                                                 u[0:2 * M, sl], sw[:])

            # ---- attention (panel-outer, head-inner) + partial projection ----
            attn_cm = tc.tile_pool(name="attn", bufs=1)
            attn = attn_cm.__enter__()
            ysb_all = attn.tile([DH, HPC, T], BF16)
            wo_sb = attn.tile([KB, HPC, C], BF16)
            for cb in range(HPC):
                nc.gpsimd.dma_start(out=wo_sb[:, cb, :], in_=wo_d[:, cb, :])

            psa_cm = tc.tile_pool(name="psa", bufs=1, space="PSUM")
            psa = psa_cm.__enter__()

            def attention_unit(J, h):
                """Emit s/exp/mask/acc/av for panel J of head h; return the
                deferred finalize closure (rowsum-MM, normalize into ysb)."""
                qh, kh = q_sb[h], k_sb[h]
                qs = J * PANEL
                nkb = (J + 1) * kb_per_panel
                py = psa.tile([DH, PANEL], F32, tag="y", bufs=2)
                acc = work.tile([KB, PANEL], BF16, tag="acc", bufs=2)
                for b in range(nkb):
                    p = b - kb_per_panel * J
                    off = max(p, 0) * KB
                    ps = psa.tile([KB, PANEL], F32, tag="s", bufs=2)
                    nc.tensor.matmul(
                        ps[:, off:],
                        kh[:, b * KB:(b + 1) * KB],
                        qh[:, qs + off:qs + PANEL],
                        start=True, stop=True)
                    et = work.tile([KB, PANEL], BF16, tag="exp", bufs=4)
                    nc.scalar.activation(
                        et[:, off:], ps[:, off:],
                        mybir.ActivationFunctionType.Exp,
                        scale=inv_sqrt_d)
                    if p >= 0:
                        nc.vector.tensor_mul(et[:, off:off + KB],
                                             et[:, off:off + KB], mask128[:])
                    if b == 0:
                        nc.vector.tensor_copy(acc[:], et[:])
                    else:
                        nc.vector.tensor_add(acc[:, off:], acc[:, off:],
                                             et[:, off:])
                    nc.tensor.matmul(
                        py[:, off:],
                        v_all[:, b, h * DH:(h + 1) * DH],
                        et[:, off:],
                        start=(b == 0), stop=(b == nkb - 1))

                def finalize():
                    pr = psa.tile([KB, PANEL], F32, tag="r", bufs=1)
                    nc.tensor.matmul(pr[:], ones128[:], acc[:],
                                     start=True, stop=True)
                    rinv = work.tile([KB, PANEL], F32, tag="rinv", bufs=2)
                    nc.vector.reciprocal_approx_fast(rinv[:], pr[:])
                    nc.vector.tensor_mul(ysb_all[:, h, qs:qs + PANEL],
                                         py[:], rinv[:])
                return finalize

            def proj_panel(J):
                """partT[:, J panel] = sum_h wo[own h].T @ y[h, J panel]."""
                ts_ = J * PANEL
                for oc in range(n_oc):
                    po = psa.tile([KB, PANEL], F32, tag="po", bufs=3)
                    for h in range(HPC):
                        nc.tensor.matmul(
                            po[:],
                            wo_sb[:, h, oc * KB:(oc + 1) * KB],
                            ysb_all[:, h, ts_:ts_ + PANEL],
                            start=(h == 0), stop=(h == HPC - 1))
                    ost = work.tile([KB, PANEL], F32, tag="ost", bufs=4)
                    if oc % 2 == 0:
                        nc.scalar.copy(ost[:], po[:])
                    else:
                        nc.vector.tensor_copy(ost[:], po[:])
                    nc.sync.dma_start(
                        out=partT_d[oc * KB:(oc + 1) * KB, ts_:ts_ + PANEL],
                        in_=ost[:])

            pending = None
            for J in range(n_panels):
                for h in range(HPC):
                    fin = attention_unit(J, h)
                    if pending is not None:
                        pending()
                    pending = fin
                    if h == 1 and J >= 1:
                        proj_panel(J - 1)
            pending()
            proj_panel(n_panels - 1)

            psa_cm.__exit__(None, None, None)
            attn_cm.__exit__(None, None, None)

    nc.compile()
    return nc


def _perm(a):
    """[C, cols] f32 -> [128, n_cb, cols] bf16 with c = kb*128 + p."""
    import ml_dtypes
    c, cols = a.shape
    return np.ascontiguousarray(
        a.reshape(c // KB, KB, cols).transpose(1, 0, 2)
    ).astype(ml_dtypes.bfloat16)


def make_in_maps(x, w_attn, w_proj, freqs, delta):
    """Host-side sharding: slice/transpose/convert full inputs per core."""
    x = np.asarray(x, dtype=np.float32)
    w_attn = np.asarray(w_attn, dtype=np.float32)
    w_proj = np.asarray(w_proj, dtype=np.float32)
    freqs = np.asarray(freqs, dtype=np.float32)
    delta = np.asarray(delta, dtype=np.float32)
    c_ = x.shape[2]
    in_maps = []
    for core in range(N_CORES):
        g, pos = divmod(core, CPG)
        heads = range(pos * HPC, (pos + 1) * HPC)
        # [qt, p, kb, 512] with c = kb*128 + p, t = qt*512 + tq
        xT = np.ascontiguousarray(
            _perm(np.ascontiguousarray(x[g].T))
            .reshape(KB, C // KB, T // PANEL, PANEL).transpose(2, 0, 1, 3))
        wqk = _perm(np.concatenate(
            [w_attn[:, h * DH:(h + 1) * DH] for h in heads]
            + [w_attn[:, c_ + h * DH:c_ + (h + 1) * DH] for h in heads],
            axis=1))
        wv = _perm(np.ascontiguousarray(
            w_attn[:, 2 * c_ + pos * C_LOC:2 * c_ + (pos + 1) * C_LOC]))
        wo = _perm(np.ascontiguousarray(
            w_proj[pos * C_LOC:(pos + 1) * C_LOC, :]))
        in_maps.append({
            "xT": xT, "wqk": wqk, "wv": wv, "wo": wo,
            "freqs": freqs, "delta": delta,
        })
    return in_maps


def assemble_output(results):
    outs = []
    for g in range(GROUPS):
        acc = results[g * CPG]["partT"].astype(np.float32)
        for pos in range(1, CPG):
            acc = acc + results[g * CPG + pos]["partT"]
        outs.append(acc.T)
    return np.stack(outs, axis=0).astype(np.float32)


_NC_CACHE = {}


def _get_program():
    if "nc" not in _NC_CACHE:
        _NC_CACHE["nc"] = build_program()
    return _NC_CACHE["nc"]


def kernel(x, w_attn, w_proj, freqs, delta):
    nc = _get_program()
    in_maps = make_in_maps(x, w_attn, w_proj, freqs, delta)
    res = run_bass_kernel_spmd(nc, in_maps, list(range(N_CORES)))
    return assemble_output(res.results)


# revision 22
# speedup vs baseline: 1.7089x; 1.0929x over previous
"""Causal self-attention with anchor-relative rope (ferope), 8-core TRN2 Bass kernel.

Full-scale problem: B=2, T=2048, C=2048, H=16, D=128, M=32.

Sharding (tensor-parallel heads + data-parallel batch), collective-free:
  - 8 cores = 2 batch groups x 4 cores. Core (b, g) handles batch b, heads 4g..4g+3.
  - All matrix inputs are pre-converted to bf16 and pre-permuted on the host so
    each contraction block [128, .] DMAs contiguously into SBUF (no staging).
  - qkv projection: per-core column shard of w_attn; q/k produced in [d, t]
    layout, v in [t, d]; rope applied per 512-panel right after projection.
  - attention runs query-panel-outer / head-inner with transposed scores
    s_T[ki, qi]; diagonal blocks narrowed to the exact causal triangle;
    softmax denominator accumulated in bf16 on the vector engine + one
    ones-matmul per panel; finalization deferred one unit to avoid stalls.
  - output projection needs no AllGather: each core computes the full-width
    partial out^T = wo_own^T @ y_own (same flops as a sharded projection,
    contraction over its own 512 channels) and the host sums the 4 partials
    per batch group while unsharding.
"""

import math

import numpy as np

import concourse.bass as bass
import concourse.mybir as mybir
import concourse.tile as tile
from concourse import bacc
from concourse.bass_utils import run_bass_kernel_spmd

F32 = mybir.dt.float32
BF16 = mybir.dt.bfloat16

# full-scale dims (hardcoded per harness contract)
B, T, C, H, DH, M = 2, 2048, 2048, 16, 128, 32
N_CORES = 8
GROUPS = 2                     # batch groups
CPG = N_CORES // GROUPS        # cores per group = 4
HPC = H // CPG                 # heads per core = 4
C_LOC = HPC * DH               # 512: per-core head channels
PANEL = 512                    # qi panel width (one psum bank)
KB = 128                       # ki block (partition dim)


def build_program():
    n_cb = C // KB              # 16 contraction blocks for qkv
    n_oc = C // KB              # 16 output-column blocks for proj
    n_panels = T // PANEL       # 4
    kb_per_panel = PANEL // KB  # 4
    inv_sqrt_d = 1.0 / math.sqrt(DH)

    nc = bacc.Bacc("TRN2", target_bir_lowering=False, debug=False,
                   num_devices=N_CORES)

    # pre-permuted bf16 inputs, contiguous per partition row so each tensor
    # loads with one big-descriptor DMA. x is quarter-major: [qt, p, kb, 512]
    xT_d = nc.dram_tensor("xT", [n_panels, KB, n_cb, PANEL], BF16,
                          kind="ExternalInput").ap()
    wqk_d = nc.dram_tensor("wqk", [KB, n_cb, 2 * C_LOC], BF16,
                           kind="ExternalInput").ap()
    wv_d = nc.dram_tensor("wv", [KB, n_cb, C_LOC], BF16,
                          kind="ExternalInput").ap()
    # proj weight rows for this core's channels: [p, own-cblk, out-cols]
    wo_d = nc.dram_tensor("wo", [KB, HPC, C], BF16, kind="ExternalInput").ap()
    freqs_d = nc.dram_tensor("freqs", [M], F32, kind="ExternalInput").ap()
    delta_d = nc.dram_tensor("delta", [T], F32, kind="ExternalInput").ap()
    # full-width transposed partial projection; host sums over the group
    partT_d = nc.dram_tensor("partT", [C, T], F32, kind="ExternalOutput").ap()

    with tile.TileContext(nc) as tc:
        with (
            tc.tile_pool(name="const", bufs=1) as const,
            tc.tile_pool(name="qkv", bufs=1) as qkv,
            tc.tile_pool(name="work", bufs=1) as work,
        ):
            # persistent attention operands
            q_sb = [qkv.tile([DH, T], BF16, name=f"q{h}") for h in range(HPC)]
            k_sb = [qkv.tile([DH, T], BF16, name=f"k{h}") for h in range(HPC)]
            v_all = qkv.tile([KB, T // KB, C_LOC], BF16)

            def rope_panel(tp):
                """Anchor-relative rope on rows 0:2M of q/k panel tp."""
                sl = slice(tp * PANEL, (tp + 1) * PANEL)
                for u in [t for pair in zip(q_sb, k_sb) for t in pair]:
                    sw = work.tile([2 * M, PANEL], BF16, tag="ropesw", bufs=2)
                    nc.vector.tensor_copy(sw[0:M, :], u[M:2 * M, sl])
                    nc.vector.tensor_copy(sw[M:2 * M, :], u[0:M, sl])
                    nc.vector.tensor_mul(sw[:], sw[:], sinN[:, sl])
                    nc.vector.tensor_mul(u[0:2 * M, sl], u[0:2 * M, sl],
                                         cos64[:, sl])
                    nc.vector.tensor_add(u[0:2 * M, sl], u[0:2 * M, sl],
                                         sw[:])

            ones128 = const.tile([KB, KB], BF16)
            sinN = const.tile([2 * M, T], F32)
            cos64 = const.tile([2 * M, T], F32)
            mask128 = const.tile([KB, KB], BF16)

            # ---- qkv projection: direct bf16 loads, per-panel rope ----
            with tc.tile_pool(name="wload", bufs=1) as wload:
                # issue the big loads FIRST (one contiguous DMA per tensor,
                # x per quarter) so nothing gates the first matmul chain
                xbf = wload.tile([KB, n_panels, n_cb, PANEL], BF16)
                wqkb = wload.tile([KB, n_cb, 2 * C_LOC], BF16)
                wvb = wload.tile([KB, n_cb, C_LOC], BF16)
                # chunk along kb and push everything from ONE queue in strict
                # priority order (ring order = push order): wv+x0 first for
                # the v chains, wqk next, later panels and wo last
                for k4 in range(0, n_cb, 4):
                    ks = slice(k4, k4 + 4)
                    nc.sync.dma_start(out=wvb[:, ks, :], in_=wv_d[:, ks, :])
                    nc.sync.dma_start(out=xbf[:, 0, ks, :],
                                      in_=xT_d[0][:, ks, :])
                for k4 in range(0, n_cb, 4):
                    ks = slice(k4, k4 + 4)
                    nc.sync.dma_start(out=wqkb[:, ks, :],
                                      in_=wqk_d[:, ks, :])
                for qt in (1, 2, 3):
                    for k4 in range(0, n_cb, 4):
                        ks = slice(k4, k4 + 4)
                        nc.sync.dma_start(out=xbf[:, qt, ks, :],
                                          in_=xT_d[qt][:, ks, :])

                # ---- constants: trig tables, diag mask, ones ----
                nc.vector.memset(ones128[:], 1.0)
                # warm the PE HAM clock gate during the initial DMA wait so
                # the first real matmuls run at full rate
                with tc.tile_pool(name="warm", bufs=1, space="PSUM") as pwarm:
                    wt = pwarm.tile([KB, KB], F32, tag="w", bufs=1)
                    for _ in range(48):
                        nc.tensor.matmul(wt[:], ones128[:], ones128[:],
                                         start=True, stop=True)
                with tc.tile_pool(name="setup", bufs=1) as setup:
                    # fr64 = [-freqs; freqs] as per-partition scalars
                    fr64 = setup.tile([2 * M, 1], F32)
                    nc.sync.dma_start(out=fr64[0:M, :],
                                      in_=freqs_d.rearrange("m -> m ()"))
                    nc.sync.dma_start(out=fr64[M:2 * M, :],
                                      in_=freqs_d.rearrange("m -> m ()"))
                    nc.vector.tensor_scalar_mul(fr64[0:M, :], fr64[0:M, :],
                                                -1.0)

                    # delta replicated across 2M partitions via 0-stride DMA
                    delta_rep = setup.tile([2 * M, T], F32)
                    nc.sync.dma_start(
                        out=delta_rep[:],
                        in_=delta_d.rearrange("t -> () t")
                        .partition_broadcast(2 * M))

                    # ang = delta * (+-freqs) in place; sinN/cos via Sin
                    nc.vector.tensor_scalar_mul(delta_rep[:], delta_rep[:],
                                                fr64[:])
                    nc.scalar.activation(sinN[:], delta_rep[:],
                                         mybir.ActivationFunctionType.Sin)
                    pi2 = setup.tile([2 * M, 1], F32)
                    nc.vector.memset(pi2[:], math.pi / 2)
                    nc.scalar.activation(cos64[:], delta_rep[:],
                                         mybir.ActivationFunctionType.Sin,
                                         bias=pi2[:])

                    # diagonal-subblock causal mask: mask[ki, c] = (c >= ki)
                    mi = setup.tile([KB, KB], F32)
                    nc.gpsimd.iota(mi[:], pattern=[[1, KB]], base=0,
                                   channel_multiplier=-1,
                                   allow_small_or_imprecise_dtypes=True)
                    nc.vector.tensor_scalar(mask128[:], mi[:], 0.0, None,
                                            mybir.AluOpType.is_ge)

                with tc.tile_pool(name="psq", bufs=1, space="PSUM") as psq:
                    for tp in range(n_panels):
                        tps = tp * PANEL
                        # v blocks for the 128-rows inside this panel
                        for tbl in range(kb_per_panel):
                            tb = tp * kb_per_panel + tbl
                            pv = psq.tile([KB, C_LOC], F32, tag="v", bufs=3)
                            for kb in range(n_cb):
                                nc.tensor.matmul(
                                    pv[:],
                                    xbf[:, tp, kb, tbl * KB:(tbl + 1) * KB],
                                    wvb[:, kb, :],
                                    start=(kb == 0), stop=(kb == n_cb - 1))
                            nc.scalar.copy(v_all[:, tb, :], pv[:])
                        # q/k column blocks: cb<HPC -> q head cb; else k head
                        for cb in range(2 * HPC):
                            pqk = psq.tile([DH, PANEL], F32, tag="qk", bufs=3)
                            for kb in range(n_cb):
                                nc.tensor.matmul(
                                    pqk[:],
                                    wqkb[:, kb, cb * DH:(cb + 1) * DH],
                                    xbf[:, tp, kb, :],
                                    start=(kb == 0), stop=(kb == n_cb - 1))
                            dst = q_sb[cb] if cb < HPC else k_sb[cb - HPC]
                            nc.scalar.copy(dst[:, tps:tps + PANEL], pqk[:])
                        # rope panels 0/1 here; 2/3 are deferred into early
                        # attention (not needed until their query panel)
                        if tp < 2:
                            rope_panel(tp)

            # ---- attention (panel-outer, head-inner) + partial projection ----
            attn_cm = tc.tile_pool(name="attn", bufs=1)
            attn = attn_cm.__enter__()
            ysb_all = attn.tile([DH, HPC, T], BF16)
            wo_sb = attn.tile([KB, HPC, C], BF16)
            for cb in range(HPC):
                nc.gpsimd.dma_start(out=wo_sb[:, cb, :], in_=wo_d[:, cb, :])

            psa_cm = tc.tile_pool(name="psa", bufs=1, space="PSUM")
            psa = psa_cm.__enter__()

            def attention_unit(J, h):
                """Emit s/exp/mask/acc/av for panel J of head h; return the
                deferred finalize closure (rowsum-MM, normalize into ysb)."""
                qh, kh = q_sb[h], k_sb[h]
                qs = J * PANEL
                nkb = (J + 1) * kb_per_panel
                py = psa.tile([DH, PANEL], F32, tag="y", bufs=2)
                acc = work.tile([KB, PANEL], BF16, tag="acc", bufs=2)
                for b in range(nkb):
                    p = b - kb_per_panel * J
                    off = max(p, 0) * KB
                    ps = psa.tile([KB, PANEL], F32, tag="s", bufs=2)
                    nc.tensor.matmul(
                        ps[:, off:],
                        kh[:, b * KB:(b + 1) * KB],
                        qh[:, qs + off:qs + PANEL],
                        start=True, stop=True)
                    et = work.tile([KB, PANEL], BF16, tag="exp", bufs=4)
                    nc.scalar.activation(
                        et[:, off:], ps[:, off:],
                        mybir.ActivationFunctionType.Exp,
                        scale=inv_sqrt_d)
                    if p >= 0:
                        nc.vector.tensor_mul(et[:, off:off + KB],
                                             et[:, off:off + KB], mask128[:])
                    if b == 0:
                        nc.vector.tensor_copy(acc[:], et[:])
                    else:
                        nc.vector.tensor_add(acc[:, off:], acc[:, off:],
                                             et[:, off:])
                    nc.tensor.matmul(
                        py[:, off:],
                        v_all[:, b, h * DH:(h + 1) * DH],
                        et[:, off:],
                        start=(b == 0), stop=(b == nkb - 1))

                def finalize():
                    pr = psa.tile([KB, PANEL], F32, tag="r", bufs=1)
                    nc.tensor.matmul(pr[:], ones128[:], acc[:],
                                     start=True, stop=True)
                    rinv = work.tile([KB, PANEL], F32, tag="rinv", bufs=2)
                    nc.vector.reciprocal_approx_fast(rinv[:], pr[:])
                    nc.vector.tensor_mul(ysb_all[:, h, qs:qs + PANEL],
                                         py[:], rinv[:])
                return finalize

            def proj_panel(J):
                """partT[:, J panel] = sum_h wo[own h].T @ y[h, J panel]."""
                ts_ = J * PANEL
                for oc in range(n_oc):
                    po = psa.tile([KB, PANEL], F32, tag="po", bufs=3)
                    for h in range(HPC):
                        nc.tensor.matmul(
                            po[:],
                            wo_sb[:, h, oc * KB:(oc + 1) * KB],
                            ysb_all[:, h, ts_:ts_ + PANEL],
                            start=(h == 0), stop=(h == HPC - 1))
                    ost = work.tile([KB, PANEL], F32, tag="ost", bufs=4)
                    if oc % 2 == 0:
                        nc.scalar.copy(ost[:], po[:])
                    else:
                        nc.vector.tensor_copy(ost[:], po[:])
                    nc.sync.dma_start(
                        out=partT_d[oc * KB:(oc + 1) * KB, ts_:ts_ + PANEL],
                        in_=ost[:])

            pending = None
            for J in range(n_panels):
                for h in range(HPC):
                    fin = attention_unit(J, h)
                    if pending is not None:
                        pending()
                    pending = fin
                    if h == 1 and J >= 1:
                        proj_panel(J - 1)
                if J < 2:
                    rope_panel(J + 2)
            pending()
            proj_panel(n_panels - 1)

            psa_cm.__exit__(None, None, None)
            attn_cm.__exit__(None, None, None)

    nc.compile()
    return nc


def _perm(a):
    """[C, cols] f32 -> [128, n_cb, cols] bf16 with c = kb*128 + p."""
    import ml_dtypes
    c, cols = a.shape
    return np.ascontiguousarray(
        a.reshape(c // KB, KB, cols).transpose(1, 0, 2)
    ).astype(ml_dtypes.bfloat16)


def make_in_maps(x, w_attn, w_proj, freqs, delta):
    """Host-side sharding: slice/transpose/convert full inputs per core."""
    x = np.asarray(x, dtype=np.float32)
    w_attn = np.asarray(w_attn, dtype=np.float32)
    w_proj = np.asarray(w_proj, dtype=np.float32)
    freqs = np.asarray(freqs, dtype=np.float32)
    delta = np.asarray(delta, dtype=np.float32)
    c_ = x.shape[2]
    in_maps = []
    for core in range(N_CORES):
        g, pos = divmod(core, CPG)
        heads = range(pos * HPC, (pos + 1) * HPC)
        # [qt, p, kb, 512] with c = kb*128 + p, t = qt*512 + tq
        xT = np.ascontiguousarray(
            _perm(np.ascontiguousarray(x[g].T))
            .reshape(KB, C // KB, T // PANEL, PANEL).transpose(2, 0, 1, 3))
        wqk = _perm(np.concatenate(
            [w_attn[:, h * DH:(h + 1) * DH] for h in heads]
            + [w_attn[:, c_ + h * DH:c_ + (h + 1) * DH] for h in heads],
            axis=1))
        wv = _perm(np.ascontiguousarray(
            w_attn[:, 2 * c_ + pos * C_LOC:2 * c_ + (pos + 1) * C_LOC]))
        wo = _perm(np.ascontiguousarray(
            w_proj[pos * C_LOC:(pos + 1) * C_LOC, :]))
        in_maps.append({
            "xT": xT, "wqk": wqk, "wv": wv, "wo": wo,
            "freqs": freqs, "delta": delta,
        })
    return in_maps


def assemble_output(results):
    outs = []
    for g in range(GROUPS):
        acc = results[g * CPG]["partT"].astype(np.float32)
        for pos in range(1, CPG):
            acc = acc + results[g * CPG + pos]["partT"]
        outs.append(acc.T)
    return np.stack(outs, axis=0).astype(np.float32)


_NC_CACHE = {}


def _get_program():
    if "nc" not in _NC_CACHE:
        _NC_CACHE["nc"] = build_program()
    return _NC_CACHE["nc"]


def kernel(x, w_attn, w_proj, freqs, delta):
    nc = _get_program()
    in_maps = make_in_maps(x, w_attn, w_proj, freqs, delta)
    res = run_bass_kernel_spmd(nc, in_maps, list(range(N_CORES)))
    return assemble_output(res.results)
